# revision 3
# baseline (speedup 1.0000x reference)
"""Multi-head attention TRN2 kernel (8 NeuronCores).

Sharding: batch (2) x head-group (4) data/tensor parallel -> 8 cores.
Core c handles batch b = c // 4 and heads [4g, 4g+4) where g = c % 4
(E-dim slice Dg = [256*g, 256*g+256)).

Device computes, per core, using transposed layouts throughout:
  QT = (WQ[Dg]/8) @ x[b].T        [256, 2048]   (1/8 = 1/sqrt(DK))
  KT = WK[Dg] @ kv[b].T           [256, 2048]
  V  = kv[b] @ WV[Dg].T           [2048, 256]  (+ ones column per head)
  ST = KT_h.T-blocks @ QT_h       [k, q] scores, transposed
  e  = exp(ST) * ebT              ebT = exp(attn_bias.T) * key-mask (host-folded)
  U' = [V_h | 1].T @ e            rows 0..63 = unnorm. head out.T, row 64 = denom
  UN = U'[0:64] / denom
  out_partial = UN.T-blocks @ WO[:, Dg].T   [2048, 1024]

Attention is processed per 512-wide query block (qb) in two head-pair
passes (heads {0,1} then {2,3}) so PSUM holds: 2 rotating score buffers
(4 banks) + the U' accumulator (2 banks) + out-proj buffers (2 banks).
AV matmuls trail the score matmuls by TRAIL kt-tiles to hide the
exp->mul latency; the out-projection of qb-1 is interleaved into qb's
score stream so the PE never drains.

Host: shards/transposes inputs, folds scale+mask+exp(bias); afterwards sums the
4 row-parallel WO partials per batch, adds WO_b, and overwrites masked-query
rows with the uniform-attention value (reference semantics for fully-masked
score rows).
"""

import math
from contextlib import ExitStack

import ml_dtypes
import numpy as np

import concourse.bass as bass
import concourse.bacc as bacc
import concourse.tile as tile
from concourse import mybir
from concourse.bass_utils import run_bass_kernel_spmd

F32 = mybir.dt.float32
F32R = mybir.dt.float32r
BF16 = mybir.dt.bfloat16
AF = mybir.ActivationFunctionType

B, S, E, H, DK = 2, 2048, 1024, 16, 64
NC = 8
DG = 256          # dims per core (4 heads x 64)
HPC = 4           # heads per core
P = 128
QB = 512          # q block
NKT = S // P      # 16 key tiles
NQB = S // QB     # 4 q blocks
NET = E // P      # 8 contraction tiles over E
TRAIL = 2         # AV matmuls trail scores by this many kt tiles

TRACE = False
LAST_RESULTS = {}

_NC_CACHE = None


def _build():
    nc = bacc.Bacc("TRN2", target_bir_lowering=False, debug=False, num_devices=NC)
    # tiled [qb, et, P, QB]: contiguous 256KB per (qb, et) block
    xT = nc.dram_tensor("xT", [NQB, NET, P, QB], F32R, kind="ExternalInput").ap()
    kvT = nc.dram_tensor("kvT", [NQB, NET, P, QB], F32R, kind="ExternalInput").ap()
    wqT = nc.dram_tensor("wqT", [E, DG], F32R, kind="ExternalInput").ap()
    wkT = nc.dram_tensor("wkT", [E, DG], F32R, kind="ExternalInput").ap()
    wvT = nc.dram_tensor("wvT", [E, DG], F32R, kind="ExternalInput").ap()
    bq = nc.dram_tensor("bq", [DG], F32, kind="ExternalInput")
    bk = nc.dram_tensor("bk", [DG], F32, kind="ExternalInput")
    bv = nc.dram_tensor("bv", [DG], F32, kind="ExternalInput")
    # tiled [qb, kt, P, QB]: contiguous per (qb, kt) block
    ebT = nc.dram_tensor("ebT", [NQB, NKT, P, QB], BF16, kind="ExternalInput").ap()
    r = nc.dram_tensor("r", [DG, E], F32R, kind="ExternalInput").ap()
    ones1 = nc.dram_tensor("ones1", [NKT * HPC * (DK + 1)], BF16, kind="ExternalInput")
    # tiled [qt2, eb, P, QB]; host reassembles
    out = nc.dram_tensor("out", [S // P, 2, P, QB], F32, kind="ExternalOutput").ap()

    with tile.TileContext(nc) as tc, ExitStack() as ctx:
        const = ctx.enter_context(tc.tile_pool(name="const", bufs=1))

        # ---- constants; DMA order matters: first matmul needs wq chunk 0
        # and xt(qb0, eg0) only, so emit those first.
        wq_sb = const.tile([P, NET, DG], F32R, name="wq_sb")
        wk_sb = const.tile([P, NET, DG], F32R, name="wk_sb")
        wv_sb = const.tile([P, NET, DG], F32R, name="wv_sb")
        for cb in range(2):
            es = slice(cb * 4, (cb + 1) * 4)
            nc.sync.dma_start(
                out=wq_sb[:, es], in_=wqT.rearrange("(t p) d -> p t d", p=P)[:, es]
            )
            nc.sync.dma_start(
                out=wk_sb[:, es], in_=wkT.rearrange("(t p) d -> p t d", p=P)[:, es]
            )
            nc.sync.dma_start(
                out=wv_sb[:, es], in_=wvT.rearrange("(t p) d -> p t d", p=P)[:, es]
            )

        vp_sb = const.tile([P, NKT, HPC, DK + 1], BF16, name="vp_sb")
        # init V' to ones; projections overwrite cols 0..DK-1 of each head
        # block, leaving col DK as the denominator-accumulator column.
        nc.sync.dma_start(
            out=vp_sb.rearrange("p a b c -> p (a b c)"),
            in_=bass.AP(tensor=ones1, offset=0, ap=[[0, P], [1, NKT * HPC * (DK + 1)]]),
        )

        bq_sb = const.tile([P, 2], F32, name="bq_sb")
        bk_sb = const.tile([P, 2], F32, name="bk_sb")
        nc.sync.dma_start(out=bq_sb, in_=bq.ap().rearrange("(t p) -> p t", p=P))
        nc.sync.dma_start(out=bk_sb, in_=bk.ap().rearrange("(t p) -> p t", p=P))
        # bv broadcast over partitions: [P, DG]
        bvb_sb = const.tile([P, DG], F32, name="bvb_sb")
        nc.sync.dma_start(
            out=bvb_sb,
            in_=bass.AP(tensor=bv, offset=0, ap=[[0, P], [1, DG]]),
        )

        r_sb = const.tile([P, 2, E], F32R, name="r_sb")
        nc.sync.dma_start(out=r_sb, in_=r.rearrange("(t p) e -> p t e", p=P))

        qt_sb = const.tile([P, 2, S], F32R, name="qt_sb")
        kt_sb = const.tile([P, 2, S], F32R, name="kt_sb")
        un_sb = const.tile([P, 2, S], F32R, name="un_sb")

        # ebT double-buffered per qb: [P, NKT, QB] bf16 (16KB/partition)
        ebpool = ctx.enter_context(tc.tile_pool(name="ebp", bufs=2))

        def emit_ebt_dma(ebt_tile, qb):
            for t2 in range(0, NKT, 2):
                nc.sync.dma_start(
                    out=ebt_tile[:, t2 : t2 + 2, :],
                    in_=ebT[qb, t2 : t2 + 2].rearrange("t p q -> p t q"),
                )

        ebt_tiles = {}
        ebt_tiles[0] = ebpool.tile([P, NKT, QB], BF16, tag="ebt", name="ebt0")
        emit_ebt_dma(ebt_tiles[0], 0)

        # ---- Phase B: projections ----
        with tc.tile_pool(name="xk", bufs=2) as xkpool, tc.tile_pool(
            name="pj_ps", bufs=1, space="PSUM"
        ) as pj:
            for qb in range(NQB):
                qs = slice(qb * QB, (qb + 1) * QB)
                ps_q = [pj.tile([P, QB], F32, tag=f"psq{d}", name=f"psq{d}") for d in range(2)]
                ps_k = [pj.tile([P, QB], F32, tag=f"psk{d}", name=f"psk{d}") for d in range(2)]
                ps_v = [pj.tile([P, DG], F32, tag=f"psv{k}", name=f"psv{k}") for k in range(4)]
                for eg in range(2):
                    xt4 = xkpool.tile([P, 4, QB], F32R, tag="xt")
                    nc.sync.dma_start(
                        out=xt4, in_=xT[qb, eg * 4 : (eg + 1) * 4].rearrange("e p q -> p e q")
                    )
                    kvt4 = xkpool.tile([P, 4, QB], F32R, tag="kvt")
                    nc.sync.dma_start(
                        out=kvt4, in_=kvT[qb, eg * 4 : (eg + 1) * 4].rearrange("e p q -> p e q")
                    )
                    for ei in range(4):
                        et = eg * 4 + ei
                        xt, kvt = xt4[:, ei], kvt4[:, ei]
                        st, sp = (et == 0), (et == NET - 1)
                        for d in range(2):
                            nc.tensor.matmul(
                                ps_q[d], wq_sb[:, et, d * P : (d + 1) * P], xt,
                                start=st, stop=sp,
                            )
                            nc.tensor.matmul(
                                ps_k[d], wk_sb[:, et, d * P : (d + 1) * P], kvt,
                                start=st, stop=sp,
                            )
                        for kb in range(4):
                            nc.tensor.matmul(
                                ps_v[kb], kvt[:, kb * P : (kb + 1) * P],
                                wv_sb[:, et, :], start=st, stop=sp,
                            )
                for d in range(2):
                    nc.vector.tensor_scalar_add(
                        qt_sb[:, d, qs], ps_q[d], bq_sb[:, d : d + 1]
                    )
                    nc.vector.tensor_scalar_add(
                        kt_sb[:, d, qs], ps_k[d], bk_sb[:, d : d + 1]
                    )
                for kb in range(4):
                    kt16 = qb * 4 + kb
                    nc.vector.tensor_add(
                        vp_sb[:, kt16, :, 0:DK],
                        ps_v[kb].rearrange("p (h d) -> p h d", h=HPC),
                        bvb_sb.rearrange("p (h d) -> p h d", h=HPC),
                    )

        # ---- Phase C: attention (two head-pair passes per qb) + interleaved
        # out-projection of qb-1 ----
        with tc.tile_pool(name="fp", bufs=3) as fpool, tc.tile_pool(
            name="ep", bufs=TRAIL + 3
        ) as epool, tc.tile_pool(name="dn", bufs=2) as dpool, tc.tile_pool(
            name="osb", bufs=3
        ) as opool, tc.tile_pool(
            name="s_ps", bufs=2, space="PSUM"
        ) as sps, tc.tile_pool(
            name="u_ps", bufs=1, space="PSUM"
        ) as ups, tc.tile_pool(name="o_ps", bufs=2, space="PSUM") as ops:

            def d_unit(qb, k):
                """Out-projection unit k (0..7) for query block qb."""
                qt2 = qb * 4 + k // 2
                eb = k % 2
                rs = slice(qt2 * P, (qt2 + 1) * P)
                es = slice(eb * QB, (eb + 1) * QB)
                ps_o = ops.tile([P, QB], F32, tag="pso")
                for d in range(2):
                    nc.tensor.matmul(
                        ps_o, un_sb[:, d, rs], r_sb[:, d, es],
                        start=(d == 0), stop=(d == 1),
                    )
                osb = opool.tile([P, QB], F32, tag="osb")
                nc.vector.tensor_copy(osb, ps_o)
                nc.sync.dma_start(out=out[qt2, eb], in_=osb)

            for qb in range(NQB):
                qs = slice(qb * QB, (qb + 1) * QB)
                ebt = ebt_tiles.pop(qb)
                unit = 0
                for hp in range(2):  # head pair: heads {2*hp, 2*hp+1}
                    ps_u = ups.tile(
                        [DK + 1, 2, QB], F32, tag="psu", name=f"psu{qb}_{hp}"
                    )
                    pend = []
                    for kt2 in range(NKT):
                        # interleave one out-proj unit of qb-1 every 4 units;
                        # offset by 3 so the first unit isn't queued before
                        # qb-1's pass-2 epilogue (recip/broadcast/mul chain)
                        # has drained — PE executes its queue in order.
                        if qb > 0 and unit % 4 == 3:
                            d_unit(qb - 1, unit // 4)
                        # prefetch next qb's ebT early in pass 0
                        if hp == 0 and kt2 == 2 and qb + 1 < NQB:
                            ebt_tiles[qb + 1] = ebpool.tile(
                                [P, NKT, QB], BF16, tag="ebt", name=f"ebt{qb+1}"
                            )
                            emit_ebt_dma(ebt_tiles[qb + 1], qb + 1)
                        ks = slice(kt2 * P, (kt2 + 1) * P)
                        ps_s = sps.tile([P, 2, QB], F32, tag="pss", name="pss")
                        for j in range(2):
                            po = j * DK
                            nc.tensor.matmul(
                                ps_s[:, j], kt_sb[po : po + DK, hp, ks],
                                qt_sb[po : po + DK, hp, qs], start=True, stop=True,
                            )
                        f2 = fpool.tile([P, 2, QB], BF16, tag="f", name="f2")
                        nc.scalar.activation(f2, ps_s, AF.Exp)
                        e2 = epool.tile([P, 2, QB], BF16, tag="e", name="e2")
                        for j in range(2):
                            nc.vector.tensor_mul(e2[:, j], f2[:, j], ebt[:, kt2, :])
                        pend.append((kt2, e2))
                        if len(pend) > TRAIL:
                            pkt, pe2 = pend.pop(0)
                            for j in range(2):
                                nc.tensor.matmul(
                                    ps_u[:, j], vp_sb[:, pkt, 2 * hp + j, :],
                                    pe2[:, j], start=(pkt == 0), stop=False,
                                )
                        unit += 1
                    for idx, (pkt, pe2) in enumerate(pend):
                        last = idx == len(pend) - 1
                        for j in range(2):
                            nc.tensor.matmul(
                                ps_u[:, j], vp_sb[:, pkt, 2 * hp + j, :],
                                pe2[:, j], start=(pkt == 0), stop=last,
                            )
                    # pass epilogue: evict U', reciprocal of denominators,
                    # broadcast along partitions, normalized write to un_sb.
                    # All off the PE critical path.
                    u_raw = dpool.tile([DK + 1, 2, QB], F32, tag="uraw", name="u_raw")
                    nc.vector.tensor_copy(u_raw, ps_u)
                    for j in range(2):
                        rd = dpool.tile([1, QB], F32, tag=f"rd{j}", name="rd")
                        nc.vector.reciprocal(rd, u_raw[DK : DK + 1, j, :])
                        rdb = dpool.tile([DK, QB], F32, tag=f"rdb{j}", name="rdb")
                        nc.gpsimd.partition_broadcast(rdb, rd)
                        nc.vector.tensor_mul(
                            un_sb[j * DK : (j + 1) * DK, hp, qs],
                            u_raw[0:DK, j, :], rdb,
                        )
            # tail: out-projection of the last qb
            for k in range(8):
                d_unit(NQB - 1, k)

    nc.compile()
    return nc


def _get_nc():
    global _NC_CACHE
    if _NC_CACHE is None:
        _NC_CACHE = _build()
    return _NC_CACHE


def kernel(x, kv, mask, attn_bias, WQ_w, WQ_b, WK_w, WK_b, WV_w, WV_b, WO_w, WO_b):
    x = np.asarray(x, dtype=np.float32)
    kv = np.asarray(kv, dtype=np.float32)
    mask = np.asarray(mask)
    attn_bias = np.asarray(attn_bias, dtype=np.float32)
    WQ_w = np.asarray(WQ_w, dtype=np.float32)
    WQ_b = np.asarray(WQ_b, dtype=np.float32)
    WK_w = np.asarray(WK_w, dtype=np.float32)
    WK_b = np.asarray(WK_b, dtype=np.float32)
    WV_w = np.asarray(WV_w, dtype=np.float32)
    WV_b = np.asarray(WV_b, dtype=np.float32)
    WO_w = np.asarray(WO_w, dtype=np.float32)
    WO_b = np.asarray(WO_b, dtype=np.float32)

    sc = 1.0 / math.sqrt(DK)
    maskf = mask.astype(np.float32)

    # per-batch host-folded tensors
    def _tile_qb(aT):
        # [E, S] -> [NQB, E//P, P, QB]
        return np.ascontiguousarray(
            aT.reshape(aT.shape[0] // P, P, NQB, QB).transpose(2, 0, 1, 3)
        )

    xTs, kvTs, ebTs = [], [], []
    for b in range(B):
        xTs.append(_tile_qb(x[b].T))
        kvTs.append(_tile_qb(kv[b].T))
        eb = (np.exp(attn_bias[b].T) * maskf[b][:, None]).astype(ml_dtypes.bfloat16)
        ebTs.append(_tile_qb(eb))

    in_maps = []
    for c in range(NC):
        b, g = c // 4, c % 4
        Dg = slice(DG * g, DG * (g + 1))
        in_maps.append(
            {
                "xT": xTs[b],
                "kvT": kvTs[b],
                "wqT": np.ascontiguousarray((WQ_w[Dg] * sc).T),
                "wkT": np.ascontiguousarray(WK_w[Dg].T),
                "wvT": np.ascontiguousarray(WV_w[Dg].T),
                "bq": np.ascontiguousarray(WQ_b[Dg] * sc),
                "bk": np.ascontiguousarray(WK_b[Dg]),
                "bv": np.ascontiguousarray(WV_b[Dg]),
                "ebT": ebTs[b],
                "r": np.ascontiguousarray(WO_w[:, Dg].T),
                "ones1": np.ones(NKT * HPC * (DK + 1), ml_dtypes.bfloat16),
            }
        )

    nc = _get_nc()
    res = run_bass_kernel_spmd(nc, in_maps, list(range(NC)), trace=TRACE)
    LAST_RESULTS["res"] = res

    out = np.zeros((B, S, E), np.float32)
    for b in range(B):
        acc = np.zeros((S, E), np.float64)
        for g in range(4):
            ot = res.results[b * 4 + g]["out"]  # [S//P, 2, P, QB]
            acc += ot.transpose(0, 2, 1, 3).reshape(S, E).astype(np.float64)
        acc += WO_b.astype(np.float64)[None, :]
        # masked-query rows: reference softmax of an all(-1e9) row is uniform
        mrows = maskf[b] == 0.0
        if mrows.any():
            meanV = (
                kv[b].astype(np.float64).mean(axis=0) @ WV_w.astype(np.float64).T
                + WV_b.astype(np.float64)
            )
            mo = meanV @ WO_w.astype(np.float64).T + WO_b.astype(np.float64)
            acc[mrows, :] = mo[None, :]
        out[b] = acc.astype(np.float32)
    return out


# revision 7
# speedup vs baseline: 1.7058x; 1.7058x over previous
"""Multi-head attention TRN2 kernel (8 NeuronCores).

Sharding: batch (2) x head-group (4) data/tensor parallel -> 8 cores.
Core c handles batch b = c // 4 and heads [4g, 4g+4) where g = c % 4
(E-dim slice Dg = [256*g, 256*g+256)).

Mask-driven compaction (host side): the reference zeroes attention
weights of masked KEYS exactly (softmax of -1e9 underflows to 0.0 in
f32), and rows for masked QUERIES are recomputed on the host (uniform
attention), so the device only processes gathered unmasked positions:
  q' = count(mask)  padded to a multiple of 128   (queries)
  k' = count(mask)  padded to a multiple of 128   (keys; zero-padded kv
       and zero ebT rows make padding exactly weightless)
For the bench mask (~50% ones) this quarters the attention work and
halves the projections.

Device computes, per core, using transposed layouts throughout:
  QT = (WQ[Dg]/8) @ xg.T          [256, q']   (1/8 = 1/sqrt(DK))
  KT = WK[Dg] @ kvg.T             [256, k']
  V  = kvg @ WV[Dg].T             [k', 256]  (+ ones column per head)
  ST = KT_h.T-blocks @ QT_h       [k, q] scores, transposed
  e  = exp(ST) * ebT              ebT = exp(attn_bias.T) gathered (host)
  U' = [V_h | 1].T @ e            rows 0..63 = unnorm. head out.T, row 64 = denom
  UN = U'[0:64] / denom
  out_partial = UN.T-blocks @ WO[:, Dg].T   [q', 1024]

Attention runs per q-block (<=512 wide) in two head-pair passes so PSUM
holds: 2 rotating score buffers (4 banks) + U' accumulator (2 banks) +
out-proj buffers (2 banks).  AV matmuls trail scores by TRAIL kt-tiles
to hide the exp->mul latency; the U'-normalization epilogue is spread
over the first units of the NEXT pass with its multiplies on the idle
GpSimd engine; the out-projection of the previous q-block interleaves
into the score stream so the PE never drains.

Host: shards/gathers/transposes inputs, folds scale+exp(bias); sums the
4 row-parallel WO partials per batch, adds WO_b, scatters rows back and
overwrites masked-query rows with the uniform-attention value.
"""

import math
from contextlib import ExitStack

import ml_dtypes
import numpy as np

import concourse.bass as bass
import concourse.bacc as bacc
import concourse.tile as tile
from concourse import mybir
from concourse.bass_utils import run_bass_kernel_spmd

F32 = mybir.dt.float32
F32R = mybir.dt.float32r
BF16 = mybir.dt.bfloat16
AF = mybir.ActivationFunctionType

B, S, E, H, DK = 2, 2048, 1024, 16, 64
NC = 8
DG = 256          # dims per core (4 heads x 64)
HPC = 4           # heads per core
P = 128
NET = E // P      # 8 contraction tiles over E
TRAIL = 2         # AV matmuls trail scores by this many kt tiles

TRACE = False
LAST_RESULTS = {}

_NC_CACHE = {}


def _blocks(n):
    """Split n chunks into blocks of 2..4 chunks (a single block may be 1)."""
    out = []
    while n > 4:
        take = 4 if n - 4 != 1 else 3
        out.append(take)
        n -= take
    out.append(n)
    return out


def _build(nq, nk):
    """nq, nk: number of 128-wide query / key chunks."""
    qblocks = _blocks(nq)
    kblocks = _blocks(nk)
    SQ, SK = nq * P, nk * P

    nc = bacc.Bacc("TRN2", target_bir_lowering=False, debug=False, num_devices=NC)
    # chunk-granular tilings: [chunk, et, P, 128]
    xT = nc.dram_tensor("xT", [nq, NET, P, P], F32R, kind="ExternalInput").ap()
    kvT = nc.dram_tensor("kvT", [nk, NET, P, P], F32R, kind="ExternalInput").ap()
    wqT = nc.dram_tensor("wqT", [E, DG], F32R, kind="ExternalInput").ap()
    wkT = nc.dram_tensor("wkT", [E, DG], F32R, kind="ExternalInput").ap()
    wvT = nc.dram_tensor("wvT", [E, DG], F32R, kind="ExternalInput").ap()
    bq = nc.dram_tensor("bq", [DG], F32, kind="ExternalInput")
    bk = nc.dram_tensor("bk", [DG], F32, kind="ExternalInput")
    bv = nc.dram_tensor("bv", [DG], F32, kind="ExternalInput")
    # [ktile, P, qchunk, 128]
    ebT = nc.dram_tensor("ebT", [nk, P, nq, P], BF16, kind="ExternalInput").ap()
    r = nc.dram_tensor("r", [DG, E], F32R, kind="ExternalInput").ap()
    ones1 = nc.dram_tensor("ones1", [nk * HPC * (DK + 1)], BF16, kind="ExternalInput")
    # [qchunk, eb, P, 512]; host reassembles
    out = nc.dram_tensor("out", [nq, 2, P, 512], F32, kind="ExternalOutput").ap()

    wqv = wqT.rearrange("(t p) d -> p t d", p=P)
    wkv = wkT.rearrange("(t p) d -> p t d", p=P)
    wvv = wvT.rearrange("(t p) d -> p t d", p=P)

    with tile.TileContext(nc) as tc, ExitStack() as ctx:
        const = ctx.enter_context(tc.tile_pool(name="const", bufs=1))

        wq_sb = const.tile([P, NET, DG], F32R, name="wq_sb")
        wk_sb = const.tile([P, NET, DG], F32R, name="wk_sb")
        wv_sb = const.tile([P, NET, DG], F32R, name="wv_sb")
        vp_sb = const.tile([P, nk, HPC, DK + 1], BF16, name="vp_sb")
        bq_sb = const.tile([P, 2], F32, name="bq_sb")
        bk_sb = const.tile([P, 2], F32, name="bk_sb")
        bvb_sb = const.tile([P, DG], F32, name="bvb_sb")
        r_sb = const.tile([P, 2, E], F32R, name="r_sb")
        qt_sb = const.tile([P, 2, SQ], F32R, name="qt_sb")
        kt_sb = const.tile([P, 2, SK], F32R, name="kt_sb")
        un_sb = const.tile([P, 2, SQ], F32R, name="un_sb")

        # ebT double-buffered per q-block: [P, nk, 512] bf16
        ebpool = ctx.enter_context(tc.tile_pool(name="ebp", bufs=2))

        def emit_ebt_dma(ebt_tile, c0, bq_):
            for kt in range(nk):
                nc.sync.dma_start(
                    out=ebt_tile[:, kt, 0 : bq_ * P],
                    in_=ebT[kt, :, c0 : c0 + bq_].rearrange("p c q -> p (c q)"),
                )

        ebt_tiles = {}
        qstart = [0]
        for bsz in qblocks:
            qstart.append(qstart[-1] + bsz)

        # ---- Phase B: projections ----
        # DMA emission order is latency-critical: first matmuls need only the
        # first weight chunks and the first kv chunk; ebT/r stream later.
        with tc.tile_pool(name="xk", bufs=4) as xkpool, tc.tile_pool(
            name="pj_ps", bufs=1, space="PSUM"
        ) as pj:
            # K/V projections over k-blocks
            c0 = 0
            for bi, bsz in enumerate(kblocks):
                ks = slice(c0 * P, (c0 + bsz) * P)
                ps_k = [pj.tile([P, 512], F32, tag=f"psk{d}", name=f"psk{d}") for d in range(2)]
                ps_v = [pj.tile([P, DG], F32, tag=f"psv{k}", name=f"psv{k}") for k in range(4)]
                for eg in range(4):
                    es2 = slice(eg * 2, (eg + 1) * 2)
                    if bi == 0:
                        nc.sync.dma_start(out=wk_sb[:, es2], in_=wkv[:, es2])
                        nc.sync.dma_start(out=wv_sb[:, es2], in_=wvv[:, es2])
                    kvt2 = xkpool.tile([P, 2, 512], F32R, tag="kvt", name="kvt2")
                    for ei in range(2):
                        nc.sync.dma_start(
                            out=kvt2[:, ei, 0 : bsz * P].rearrange(
                                "p (c q) -> p c q", c=bsz
                            ),
                            in_=kvT[c0 : c0 + bsz, eg * 2 + ei].rearrange(
                                "c p q -> p c q"
                            ),
                        )
                    if bi == 0 and eg == 0:
                        nc.sync.dma_start(
                            out=bq_sb, in_=bq.ap().rearrange("(t p) -> p t", p=P)
                        )
                        nc.sync.dma_start(
                            out=bk_sb, in_=bk.ap().rearrange("(t p) -> p t", p=P)
                        )
                        nc.sync.dma_start(
                            out=bvb_sb,
                            in_=bass.AP(tensor=bv, offset=0, ap=[[0, P], [1, DG]]),
                        )
                        nc.sync.dma_start(
                            out=vp_sb.rearrange("p a b c -> p (a b c)"),
                            in_=bass.AP(
                                tensor=ones1, offset=0,
                                ap=[[0, P], [1, nk * HPC * (DK + 1)]],
                            ),
                        )
                    for ei in range(2):
                        et = eg * 2 + ei
                        kvt = kvt2[:, ei, 0 : bsz * P]
                        st, sp = (et == 0), (et == NET - 1)
                        for d in range(2):
                            nc.tensor.matmul(
                                ps_k[d][:, 0 : bsz * P],
                                wk_sb[:, et, d * P : (d + 1) * P], kvt,
                                start=st, stop=sp,
                            )
                        for kb in range(bsz):
                            nc.tensor.matmul(
                                ps_v[kb], kvt[:, kb * P : (kb + 1) * P],
                                wv_sb[:, et, :], start=st, stop=sp,
                            )
                for d in range(2):
                    nc.vector.tensor_scalar_add(
                        kt_sb[:, d, ks], ps_k[d][:, 0 : bsz * P], bk_sb[:, d : d + 1]
                    )
                for kb in range(bsz):
                    nc.vector.tensor_add(
                        vp_sb[:, c0 + kb, :, 0:DK],
                        ps_v[kb].rearrange("p (h d) -> p h d", h=HPC),
                        bvb_sb.rearrange("p (h d) -> p h d", h=HPC),
                    )
                c0 += bsz
            # Q projections over q-blocks
            c0 = 0
            for bi, bsz in enumerate(qblocks):
                qs = slice(c0 * P, (c0 + bsz) * P)
                ps_q = [pj.tile([P, 512], F32, tag=f"psq{d}", name=f"psq{d}") for d in range(2)]
                for eg in range(4):
                    es2 = slice(eg * 2, (eg + 1) * 2)
                    if bi == 0:
                        nc.sync.dma_start(out=wq_sb[:, es2], in_=wqv[:, es2])
                    xt2 = xkpool.tile([P, 2, 512], F32R, tag="xt", name="xt2")
                    for ei in range(2):
                        nc.sync.dma_start(
                            out=xt2[:, ei, 0 : bsz * P].rearrange(
                                "p (c q) -> p c q", c=bsz
                            ),
                            in_=xT[c0 : c0 + bsz, eg * 2 + ei].rearrange(
                                "c p q -> p c q"
                            ),
                        )
                    for ei in range(2):
                        et = eg * 2 + ei
                        xt = xt2[:, ei, 0 : bsz * P]
                        st, sp = (et == 0), (et == NET - 1)
                        for d in range(2):
                            nc.tensor.matmul(
                                ps_q[d][:, 0 : bsz * P],
                                wq_sb[:, et, d * P : (d + 1) * P], xt,
                                start=st, stop=sp,
                            )
                for d in range(2):
                    nc.vector.tensor_scalar_add(
                        qt_sb[:, d, qs], ps_q[d][:, 0 : bsz * P], bq_sb[:, d : d + 1]
                    )
                if bi == 0:
                    ebt_tiles[0] = ebpool.tile([P, nk, 512], BF16, tag="ebt", name="ebt0")
                    emit_ebt_dma(ebt_tiles[0], 0, qblocks[0])
                    nc.sync.dma_start(
                        out=r_sb, in_=r.rearrange("(t p) e -> p t e", p=P)
                    )
                c0 += bsz

        # ---- Phase C: attention + interleaved out-projection ----
        with tc.tile_pool(name="fp", bufs=3) as fpool, tc.tile_pool(
            name="ep", bufs=TRAIL + 3
        ) as epool, tc.tile_pool(name="dn", bufs=2) as dpool, tc.tile_pool(
            name="osb", bufs=3
        ) as opool, tc.tile_pool(
            name="s_ps", bufs=2, space="PSUM"
        ) as sps, tc.tile_pool(
            name="u_ps", bufs=1, space="PSUM"
        ) as ups, tc.tile_pool(name="o_ps", bufs=2, space="PSUM") as ops:

            def d_unit(qchunk, eb):
                """Out-projection for one (128-query chunk, 512-col half)."""
                rs = slice(qchunk * P, (qchunk + 1) * P)
                es = slice(eb * 512, (eb + 1) * 512)
                ps_o = ops.tile([P, 512], F32, tag="pso", name="pso")
                for d in range(2):
                    nc.tensor.matmul(
                        ps_o, un_sb[:, d, rs], r_sb[:, d, es],
                        start=(d == 0), stop=(d == 1),
                    )
                osb = opool.tile([P, 512], F32, tag="osb", name="osb")
                nc.vector.tensor_copy(osb, ps_o)
                nc.sync.dma_start(out=out[qchunk, eb], in_=osb)

            def epi_step(st, step):
                """One piece of a finished pass's deferred epilogue.
                recip on DVE; broadcast + normalize-mul on GpSimd so the DVE
                queue never waits on the cross-engine chain."""
                j = 0 if step < 2 else 1
                if step % 2 == 0:
                    rd = dpool.tile([1, 512], F32, tag=f"rd{j}", name="rd")
                    nc.vector.reciprocal(
                        rd[:, 0 : st["w"]], st["u"][DK : DK + 1, j, 0 : st["w"]]
                    )
                    rdb = dpool.tile([DK, 512], F32, tag=f"rdb{j}", name="rdb")
                    nc.gpsimd.partition_broadcast(
                        rdb[:, 0 : st["w"]], rd[:, 0 : st["w"]]
                    )
                    st["rdb"] = rdb
                else:
                    rdb = st.pop("rdb")
                    nc.gpsimd.tensor_mul(
                        un_sb[j * DK : (j + 1) * DK, st["hp"], st["qs"]],
                        st["u"][0:DK, j, 0 : st["w"]], rdb[:, 0 : st["w"]],
                    )

            epi_pend = None  # (state dict, emitted-steps)
            d_pend = []      # (qchunk, eb) out-proj units awaiting a slot

            for qi, bsz in enumerate(qblocks):
                c0 = qstart[qi]
                w = bsz * P
                qs = slice(c0 * P, (c0 + bsz) * P)
                ebt = ebt_tiles.pop(qi)
                for hp in range(2):  # head pair: heads {2*hp, 2*hp+1}
                    ps_u = ups.tile([DK + 1, 2, 512], F32, tag="psu", name="psu")
                    pend = []
                    for kt2 in range(nk):
                        if epi_pend is not None and kt2 < 4:
                            epi_step(epi_pend, kt2)
                            if kt2 == 3:
                                epi_pend = None
                        elif d_pend and kt2 >= 4:
                            d_unit(*d_pend.pop(0))
                        if hp == 0 and kt2 == 2 and qi + 1 < len(qblocks):
                            ebt_tiles[qi + 1] = ebpool.tile(
                                [P, nk, 512], BF16, tag="ebt", name="ebt1"
                            )
                            emit_ebt_dma(ebt_tiles[qi + 1], qstart[qi + 1], qblocks[qi + 1])
                        ks = slice(kt2 * P, (kt2 + 1) * P)
                        ps_s = sps.tile([P, 2, 512], F32, tag="pss", name="pss")
                        for j in range(2):
                            po = j * DK
                            nc.tensor.matmul(
                                ps_s[:, j, 0:w], kt_sb[po : po + DK, hp, ks],
                                qt_sb[po : po + DK, hp, qs], start=True, stop=True,
                            )
                        f2 = fpool.tile([P, 2, 512], BF16, tag="f", name="f2")
                        nc.scalar.activation(
                            f2[:, :, 0:w], ps_s[:, :, 0:w], AF.Exp
                        )
                        e2 = epool.tile([P, 2, 512], BF16, tag="e", name="e2")
                        for j in range(2):
                            nc.vector.tensor_mul(
                                e2[:, j, 0:w], f2[:, j, 0:w], ebt[:, kt2, 0:w]
                            )
                        pend.append((kt2, e2))
                        if len(pend) > TRAIL:
                            pkt, pe2 = pend.pop(0)
                            for j in range(2):
                                nc.tensor.matmul(
                                    ps_u[:, j, 0:w], vp_sb[:, pkt, 2 * hp + j, :],
                                    pe2[:, j, 0:w], start=(pkt == 0), stop=False,
                                )
                    for idx, (pkt, pe2) in enumerate(pend):
                        last = idx == len(pend) - 1
                        for j in range(2):
                            nc.tensor.matmul(
                                ps_u[:, j, 0:w], vp_sb[:, pkt, 2 * hp + j, :],
                                pe2[:, j, 0:w], start=(pkt == 0), stop=last,
                            )
                    # evict U' now (frees the PSUM accumulator); the rest of
                    # the epilogue is spread over the next pass's units.
                    u_raw = dpool.tile([DK + 1, 2, 512], F32, tag="uraw", name="u_raw")
                    nc.vector.tensor_copy(u_raw[:, :, 0:w], ps_u[:, :, 0:w])
                    if epi_pend is not None:  # nk < 4: flush leftovers
                        for step in range(4):
                            epi_step(epi_pend, step)
                    epi_pend = {"u": u_raw, "hp": hp, "qs": qs, "w": w}
                    if hp == 1:
                        d_pend.extend((c0 + c, eb) for c in range(bsz) for eb in range(2))
            # tail: last pass's epilogue + remaining out-proj units
            for step in range(4):
                epi_step(epi_pend, step)
            for unit in d_pend:
                d_unit(*unit)

    nc.compile()
    return nc


def _get_nc(nq, nk):
    key = (nq, nk)
    if key not in _NC_CACHE:
        _NC_CACHE[key] = _build(nq, nk)
    return _NC_CACHE[key]


def kernel(x, kv, mask, attn_bias, WQ_w, WQ_b, WK_w, WK_b, WV_w, WV_b, WO_w, WO_b):
    x = np.asarray(x, dtype=np.float32)
    kv = np.asarray(kv, dtype=np.float32)
    mask = np.asarray(mask)
    attn_bias = np.asarray(attn_bias, dtype=np.float32)
    WQ_w = np.asarray(WQ_w, dtype=np.float32)
    WQ_b = np.asarray(WQ_b, dtype=np.float32)
    WK_w = np.asarray(WK_w, dtype=np.float32)
    WK_b = np.asarray(WK_b, dtype=np.float32)
    WV_w = np.asarray(WV_w, dtype=np.float32)
    WV_b = np.asarray(WV_b, dtype=np.float32)
    WO_w = np.asarray(WO_w, dtype=np.float32)
    WO_b = np.asarray(WO_b, dtype=np.float32)

    sc = 1.0 / math.sqrt(DK)
    maskf = mask.astype(np.float32)

    idxs = [np.nonzero(mask[b])[0] for b in range(B)]
    counts = [len(ix) for ix in idxs]
    nq = nk = max(1, max((c + P - 1) // P for c in counts))
    SQ = SK = nq * P

    def _tile_chunks(aT, n):
        # [E, n*P] -> [n, E//P, P, P]
        return np.ascontiguousarray(
            aT.reshape(NET, P, n, P).transpose(2, 0, 1, 3)
        )

    xTs, kvTs, ebTs = [], [], []
    for b in range(B):
        ix = idxs[b]
        xg = np.zeros((SQ, E), np.float32)
        xg[: counts[b]] = x[b][ix]
        kvg = np.zeros((SK, E), np.float32)
        kvg[: counts[b]] = kv[b][ix]
        ebg = np.zeros((SK, SQ), np.float32)
        ebg[: counts[b], : counts[b]] = np.exp(attn_bias[b][np.ix_(ix, ix)]).T
        xTs.append(_tile_chunks(xg.T, nq))
        kvTs.append(_tile_chunks(kvg.T, nk))
        # [SK, SQ] -> [nk, P, nq, P]
        ebTs.append(
            np.ascontiguousarray(
                ebg.astype(ml_dtypes.bfloat16)
                .reshape(nk, P, nq, P)
            )
        )

    in_maps = []
    for c in range(NC):
        b, g = c // 4, c % 4
        Dg = slice(DG * g, DG * (g + 1))
        in_maps.append(
            {
                "xT": xTs[b],
                "kvT": kvTs[b],
                "wqT": np.ascontiguousarray((WQ_w[Dg] * sc).T),
                "wkT": np.ascontiguousarray(WK_w[Dg].T),
                "wvT": np.ascontiguousarray(WV_w[Dg].T),
                "bq": np.ascontiguousarray(WQ_b[Dg] * sc),
                "bk": np.ascontiguousarray(WK_b[Dg]),
                "bv": np.ascontiguousarray(WV_b[Dg]),
                "ebT": ebTs[b],
                "r": np.ascontiguousarray(WO_w[:, Dg].T),
                "ones1": np.ones(nk * HPC * (DK + 1), ml_dtypes.bfloat16),
            }
        )

    nc = _get_nc(nq, nk)
    res = run_bass_kernel_spmd(nc, in_maps, list(range(NC)), trace=TRACE)
    LAST_RESULTS["res"] = res

    out = np.zeros((B, S, E), np.float32)
    for b in range(B):
        acc = np.zeros((SQ, E), np.float64)
        for g in range(4):
            ot = res.results[b * 4 + g]["out"]  # [nq, 2, P, 512]
            acc += ot.transpose(0, 2, 1, 3).reshape(SQ, E).astype(np.float64)
        acc += WO_b.astype(np.float64)[None, :]
        full = np.zeros((S, E), np.float64)
        full[idxs[b]] = acc[: counts[b]]
        # masked-query rows: reference softmax of an all(-1e9) row is uniform
        mrows = maskf[b] == 0.0
        if mrows.any():
            meanV = (
                kv[b].astype(np.float64).mean(axis=0) @ WV_w.astype(np.float64).T
                + WV_b.astype(np.float64)
            )
            mo = meanV @ WO_w.astype(np.float64).T + WO_b.astype(np.float64)
            full[mrows, :] = mo[None, :]
        out[b] = full.astype(np.float32)
    return out


# revision 12
# speedup vs baseline: 2.0595x; 1.2074x over previous
"""Multi-head attention TRN2 kernel (8 NeuronCores).

Sharding: batch (2) x head-group (4) data/tensor parallel -> 8 cores.
Core c handles batch b = c // 4 and heads [4g, 4g+4) where g = c % 4
(E-dim slice Dg = [256*g, 256*g+256)).

Mask-driven compaction (host side): the reference zeroes attention
weights of masked KEYS exactly (softmax of -1e9 underflows to 0.0 in
f32), and rows for masked QUERIES are recomputed on the host (uniform
attention), so the device only processes gathered unmasked positions:
  q' = count(mask)  padded to a multiple of 128   (queries)
  k' = count(mask)  padded to a multiple of 128   (keys; zero-padded kv
       and zero ebT rows make padding exactly weightless)
For the bench mask (~50% ones) this quarters the attention work and
halves the projections.

Device computes, per core, using transposed layouts throughout:
  QT = (WQ[Dg]/8) @ xg.T          [256, q']   (1/8 = 1/sqrt(DK))
  KT = WK[Dg] @ kvg.T             [256, k']
  V  = kvg @ WV[Dg].T             [k', 256]  (+ ones column per head)
  ST = KT_h.T-blocks @ QT_h       [k, q] scores, transposed
  e  = exp(ST) * ebT              ebT = exp(attn_bias.T) gathered (host)
  U' = [V_h | 1].T @ e            rows 0..63 = unnorm. head out.T, row 64 = denom
  UN = U'[0:64] / denom
  out_partial = UN.T-blocks @ WO[:, Dg].T   [q', 1024]

Attention runs per q-block (<=512 wide) in two head-pair passes so PSUM
holds: 2 rotating score buffers (4 banks) + U' accumulator (2 banks) +
out-proj buffers (2 banks).  AV matmuls trail scores by TRAIL kt-tiles
to hide the exp->mul latency; the U'-normalization epilogue is spread
over the first units of the NEXT pass with its multiplies on the idle
GpSimd engine; the out-projection of the previous q-block interleaves
into the score stream so the PE never drains.

Host: shards/gathers/transposes inputs, folds scale+exp(bias); sums the
4 row-parallel WO partials per batch, adds WO_b, scatters rows back and
overwrites masked-query rows with the uniform-attention value.
"""

import math
from contextlib import ExitStack

import ml_dtypes
import numpy as np

import concourse.bass as bass
import concourse.bacc as bacc
import concourse.tile as tile
from concourse import mybir
from concourse.bass_utils import run_bass_kernel_spmd

F32 = mybir.dt.float32
F32R = mybir.dt.float32r
BF16 = mybir.dt.bfloat16
AF = mybir.ActivationFunctionType

B, S, E, H, DK = 2, 2048, 1024, 16, 64
NC = 8
DG = 256          # dims per core (4 heads x 64)
HPC = 4           # heads per core
P = 128
NET = E // P      # 8 contraction tiles over E
TRAIL = 2         # AV matmuls trail scores by this many kt tiles

TRACE = False
LAST_RESULTS = {}

_NC_CACHE = {}


def _blocks(n):
    """Split n chunks into blocks of 2..4 chunks (a single block may be 1)."""
    out = []
    while n > 4:
        take = 4 if n - 4 != 1 else 3
        out.append(take)
        n -= take
    out.append(n)
    return out


def _build(nq, nk):
    """nq, nk: number of 128-wide query / key chunks."""
    qblocks = _blocks(nq)
    kblocks = _blocks(nk)
    SQ, SK = nq * P, nk * P

    nc = bacc.Bacc("TRN2", target_bir_lowering=False, debug=False, num_devices=NC)
    # chunk-granular tilings: [chunk, et, P, 128]
    xT = nc.dram_tensor("xT", [nq, NET, P, P], F32R, kind="ExternalInput").ap()
    kvT = nc.dram_tensor("kvT", [nk, NET, P, P], F32R, kind="ExternalInput").ap()
    wqT = nc.dram_tensor("wqT", [E, DG], F32R, kind="ExternalInput").ap()
    wkT = nc.dram_tensor("wkT", [E, DG], F32R, kind="ExternalInput").ap()
    wvT = nc.dram_tensor("wvT", [E, DG], F32R, kind="ExternalInput").ap()
    bq = nc.dram_tensor("bq", [DG], F32, kind="ExternalInput")
    bk = nc.dram_tensor("bk", [DG], F32, kind="ExternalInput")
    bv = nc.dram_tensor("bv", [DG], F32, kind="ExternalInput")
    # [ktile, P, qchunk, 128]
    ebT = nc.dram_tensor("ebT", [nk, P, nq, P], BF16, kind="ExternalInput").ap()
    r = nc.dram_tensor("r", [DG, E], F32R, kind="ExternalInput").ap()
    ones1 = nc.dram_tensor("ones1", [nk * HPC * (DK + 1)], BF16, kind="ExternalInput")
    onespe = nc.dram_tensor("onespe", [DK], F32R, kind="ExternalInput")
    # [qchunk, eb, P, 512]; host reassembles
    out = nc.dram_tensor("out", [nq, 2, P, 512], F32, kind="ExternalOutput").ap()

    wqv = wqT.rearrange("(t p) d -> p t d", p=P)
    wkv = wkT.rearrange("(t p) d -> p t d", p=P)
    wvv = wvT.rearrange("(t p) d -> p t d", p=P)

    with tile.TileContext(nc) as tc, ExitStack() as ctx:
        const = ctx.enter_context(tc.tile_pool(name="const", bufs=1))

        wq_sb = const.tile([P, NET, DG], F32R, name="wq_sb")
        wk_sb = const.tile([P, NET, DG], F32R, name="wk_sb")
        wv_sb = const.tile([P, NET, DG], F32R, name="wv_sb")
        vp_sb = const.tile([P, nk, HPC, DK + 1], BF16, name="vp_sb")
        bq_sb = const.tile([P, 2], F32, name="bq_sb")
        bk_sb = const.tile([P, 2], F32, name="bk_sb")
        bvb_sb = const.tile([P, DG], F32, name="bvb_sb")
        r_sb = const.tile([P, 2, E], F32R, name="r_sb")
        qt_sb = const.tile([P, 2, SQ], F32R, name="qt_sb")
        kt_sb = const.tile([P, 2, SK], F32R, name="kt_sb")
        un_sb = const.tile([P, 2, SQ], F32R, name="un_sb")
        onespe_sb = const.tile([1, DK], F32R, name="onespe_sb")

        # ebT double-buffered per q-block: [P, nk, 512] bf16
        ebpool = ctx.enter_context(tc.tile_pool(name="ebp", bufs=2))

        def emit_ebt_dma(ebt_tile, c0, bq_):
            for kt in range(nk):
                nc.sync.dma_start(
                    out=ebt_tile[:, kt, 0 : bq_ * P],
                    in_=ebT[kt, :, c0 : c0 + bq_].rearrange("p c q -> p (c q)"),
                )

        ebt_tiles = {}
        qstart = [0]
        for bsz in qblocks:
            qstart.append(qstart[-1] + bsz)

        # ---- Phase B: projections ----
        # DMA emission order is latency-critical: first matmuls need only the
        # first weight chunks and the first kv chunk; ebT/r stream later.
        with tc.tile_pool(name="xk", bufs=4) as xkpool, tc.tile_pool(
            name="pj_ps", bufs=1, space="PSUM"
        ) as pj:
            # K/V projections over k-blocks
            c0 = 0
            for bi, bsz in enumerate(kblocks):
                ks = slice(c0 * P, (c0 + bsz) * P)
                ps_k = [pj.tile([P, 512], F32, tag=f"psk{d}", name=f"psk{d}") for d in range(2)]
                ps_v = [pj.tile([P, DG], F32, tag=f"psv{k}", name=f"psv{k}") for k in range(4)]
                kvts = []
                for eg in range(4):
                    es2 = slice(eg * 2, (eg + 1) * 2)
                    if bi == 0:
                        nc.sync.dma_start(out=wk_sb[:, es2], in_=wkv[:, es2])
                        nc.sync.dma_start(out=wv_sb[:, es2], in_=wvv[:, es2])
                    kvt2 = xkpool.tile([P, 2, 512], F32R, tag="kvt", name="kvt2")
                    kvts.append(kvt2)
                    for ei in range(2):
                        nc.sync.dma_start(
                            out=kvt2[:, ei, 0 : bsz * P].rearrange(
                                "p (c q) -> p c q", c=bsz
                            ),
                            in_=kvT[c0 : c0 + bsz, eg * 2 + ei].rearrange(
                                "c p q -> p c q"
                            ),
                        )
                    if bi == 0 and eg == 0:
                        nc.sync.dma_start(
                            out=bq_sb, in_=bq.ap().rearrange("(t p) -> p t", p=P)
                        )
                        nc.sync.dma_start(
                            out=bk_sb, in_=bk.ap().rearrange("(t p) -> p t", p=P)
                        )
                        nc.sync.dma_start(
                            out=bvb_sb,
                            in_=bass.AP(tensor=bv, offset=0, ap=[[0, P], [1, DG]]),
                        )
                        nc.sync.dma_start(
                            out=vp_sb.rearrange("p a b c -> p (a b c)"),
                            in_=bass.AP(
                                tensor=ones1, offset=0,
                                ap=[[0, P], [1, nk * HPC * (DK + 1)]],
                            ),
                        )
                        nc.sync.dma_start(
                            out=onespe_sb,
                            in_=bass.AP(tensor=onespe, offset=0, ap=[[0, 1], [1, DK]]),
                        )
                for eg in range(4):
                    kvt2 = kvts[eg]
                    for ei in range(2):
                        et = eg * 2 + ei
                        kvt = kvt2[:, ei, 0 : bsz * P]
                        st, sp = (et == 0), (et == NET - 1)
                        for d in range(2):
                            nc.tensor.matmul(
                                ps_k[d][:, 0 : bsz * P],
                                wk_sb[:, et, d * P : (d + 1) * P], kvt,
                                start=st, stop=sp,
                            )
                        for kb in range(bsz):
                            nc.tensor.matmul(
                                ps_v[kb], kvt[:, kb * P : (kb + 1) * P],
                                wv_sb[:, et, :], start=st, stop=sp,
                            )
                for d in range(2):
                    nc.vector.tensor_scalar_add(
                        kt_sb[:, d, ks], ps_k[d][:, 0 : bsz * P], bk_sb[:, d : d + 1]
                    )
                for kb in range(bsz):
                    nc.vector.tensor_add(
                        vp_sb[:, c0 + kb, :, 0:DK],
                        ps_v[kb].rearrange("p (h d) -> p h d", h=HPC),
                        bvb_sb.rearrange("p (h d) -> p h d", h=HPC),
                    )
                if bi == 0:
                    ebt_tiles[0] = ebpool.tile([P, nk, 512], BF16, tag="ebt", name="ebt0")
                    emit_ebt_dma(ebt_tiles[0], 0, qblocks[0])
                    nc.sync.dma_start(
                        out=r_sb, in_=r.rearrange("(t p) e -> p t e", p=P)
                    )
                c0 += bsz
            # Q projections over q-blocks
            c0 = 0
            for bi, bsz in enumerate(qblocks):
                qs = slice(c0 * P, (c0 + bsz) * P)
                ps_q = [pj.tile([P, 512], F32, tag=f"psq{d}", name=f"psq{d}") for d in range(2)]
                xts = []
                for eg in range(4):
                    es2 = slice(eg * 2, (eg + 1) * 2)
                    if bi == 0:
                        nc.sync.dma_start(out=wq_sb[:, es2], in_=wqv[:, es2])
                    xt2 = xkpool.tile([P, 2, 512], F32R, tag="xt", name="xt2")
                    xts.append(xt2)
                    for ei in range(2):
                        nc.sync.dma_start(
                            out=xt2[:, ei, 0 : bsz * P].rearrange(
                                "p (c q) -> p c q", c=bsz
                            ),
                            in_=xT[c0 : c0 + bsz, eg * 2 + ei].rearrange(
                                "c p q -> p c q"
                            ),
                        )
                for eg in range(4):
                    xt2 = xts[eg]
                    for ei in range(2):
                        et = eg * 2 + ei
                        xt = xt2[:, ei, 0 : bsz * P]
                        st, sp = (et == 0), (et == NET - 1)
                        for d in range(2):
                            nc.tensor.matmul(
                                ps_q[d][:, 0 : bsz * P],
                                wq_sb[:, et, d * P : (d + 1) * P], xt,
                                start=st, stop=sp,
                            )
                for d in range(2):
                    nc.vector.tensor_scalar_add(
                        qt_sb[:, d, qs], ps_q[d][:, 0 : bsz * P], bq_sb[:, d : d + 1]
                    )
                c0 += bsz

        # ---- Phase C: attention + interleaved out-projection ----
        with tc.tile_pool(name="fp", bufs=3) as fpool, tc.tile_pool(
            name="ep", bufs=TRAIL + 3
        ) as epool, tc.tile_pool(name="dn", bufs=2) as dpool, tc.tile_pool(
            name="osb", bufs=3
        ) as opool, tc.tile_pool(
            name="s_ps", bufs=2, space="PSUM"
        ) as sps, tc.tile_pool(
            name="u_ps", bufs=1, space="PSUM"
        ) as ups, tc.tile_pool(name="o_ps", bufs=1, space="PSUM") as ops, tc.tile_pool(
            name="b_ps", bufs=1, space="PSUM"
        ) as bps:

            def d_unit(qchunk, eb):
                """Out-projection for one (128-query chunk, 512-col half)."""
                rs = slice(qchunk * P, (qchunk + 1) * P)
                es = slice(eb * 512, (eb + 1) * 512)
                ps_o = ops.tile([P, 512], F32, tag="pso", name="pso")
                for d in range(2):
                    nc.tensor.matmul(
                        ps_o, un_sb[:, d, rs], r_sb[:, d, es],
                        start=(d == 0), stop=(d == 1),
                    )
                osb = opool.tile([P, 512], F32, tag="osb", name="osb")
                if eb == 0:
                    nc.scalar.activation(osb, ps_o, AF.Copy)
                else:
                    nc.vector.tensor_copy(osb, ps_o)
                nc.sync.dma_start(out=out[qchunk, eb], in_=osb)

            def epi_step(st, step):
                """One piece of a finished pass's deferred epilogue, spread
                over the next pass so no engine queue blocks for long:
                approx-reciprocal of the denominators (DVE), partition
                broadcast via a K=1 PE matmul (ones^T x recip -> PSUM), then
                the normalizing multiplies (DVE)."""
                w, hp, qs, u = st["w"], st["hp"], st["qs"], st["u"]
                if step == 0:
                    rd = dpool.tile([1, 2, 512], F32, tag="rd", name="rd")
                    nc.vector.reciprocal(
                        rd[:, :, 0:w], u[DK : DK + 1, :, 0:w]
                    )
                    st["rd"] = rd
                elif step in (1, 2):
                    j = step - 1
                    rdb = dpool.tile([DK, 512], F32, tag="rdb", name="rdb")
                    nc.gpsimd.partition_broadcast(
                        rdb[:, 0:w], st["rd"][0:1, j, 0:w]
                    )
                    nc.vector.tensor_mul(
                        un_sb[j * DK : (j + 1) * DK, hp, qs],
                        u[0:DK, j, 0:w], rdb[:, 0:w],
                    )

            epi_pend = None  # (state dict, emitted-steps)
            d_pend = []      # (qchunk, eb) out-proj units awaiting a slot

            for qi, bsz in enumerate(qblocks):
                c0 = qstart[qi]
                w = bsz * P
                qs = slice(c0 * P, (c0 + bsz) * P)
                ebt = ebt_tiles.pop(qi)
                for hp in range(2):  # head pair: heads {2*hp, 2*hp+1}
                    ps_u = ups.tile([DK + 1, 2, 512], F32, tag="psu", name="psu")
                    pend = []
                    for kt2 in range(nk):
                        if epi_pend is not None and kt2 in (0, 2, 3):
                            epi_step(epi_pend, 0 if kt2 == 0 else kt2 - 1)
                            if kt2 == 3:
                                epi_pend = None
                        if d_pend and kt2 >= 4:
                            d_unit(*d_pend.pop(0))
                        if hp == 0 and kt2 == 2 and qi + 1 < len(qblocks):
                            ebt_tiles[qi + 1] = ebpool.tile(
                                [P, nk, 512], BF16, tag="ebt", name="ebt1"
                            )
                            emit_ebt_dma(ebt_tiles[qi + 1], qstart[qi + 1], qblocks[qi + 1])
                        ks = slice(kt2 * P, (kt2 + 1) * P)
                        ps_s = sps.tile([P, 2, 512], F32, tag="pss", name="pss")
                        for j in range(2):
                            po = j * DK
                            nc.tensor.matmul(
                                ps_s[:, j, 0:w], kt_sb[po : po + DK, hp, ks],
                                qt_sb[po : po + DK, hp, qs], start=True, stop=True,
                            )
                        f2 = fpool.tile([P, 2, 512], BF16, tag="f", name="f2")
                        nc.scalar.activation(
                            f2[:, :, 0:w], ps_s[:, :, 0:w], AF.Exp
                        )
                        e2 = epool.tile([P, 2, 512], BF16, tag="e", name="e2")
                        for j in range(2):
                            nc.vector.tensor_mul(
                                e2[:, j, 0:w], f2[:, j, 0:w], ebt[:, kt2, 0:w]
                            )
                        pend.append((kt2, e2))
                        if len(pend) > TRAIL:
                            pkt, pe2 = pend.pop(0)
                            for j in range(2):
                                nc.tensor.matmul(
                                    ps_u[:, j, 0:w], vp_sb[:, pkt, 2 * hp + j, :],
                                    pe2[:, j, 0:w], start=(pkt == 0), stop=False,
                                )
                    for idx, (pkt, pe2) in enumerate(pend):
                        last = idx == len(pend) - 1
                        for j in range(2):
                            nc.tensor.matmul(
                                ps_u[:, j, 0:w], vp_sb[:, pkt, 2 * hp + j, :],
                                pe2[:, j, 0:w], start=(pkt == 0), stop=last,
                            )
                    # evict U' now (frees the PSUM accumulator); the rest of
                    # the epilogue is spread over the next pass's units.
                    u_raw = dpool.tile([DK + 1, 2, 512], F32, tag="uraw", name="u_raw")
                    nc.scalar.activation(u_raw[:, :, 0:w], ps_u[:, :, 0:w], AF.Copy)
                    if epi_pend is not None:  # nk < 4: flush leftovers
                        for step in range(3):
                            epi_step(epi_pend, step)
                    epi_pend = {"u": u_raw, "hp": hp, "qs": qs, "w": w}
                    if hp == 1:
                        d_pend.extend((c0 + c, eb) for c in range(bsz) for eb in range(2))
            # tail: last pass's epilogue + remaining out-proj units
            for step in range(3):
                epi_step(epi_pend, step)
            for unit in d_pend:
                d_unit(*unit)

    nc.compile()
    return nc


def _get_nc(nq, nk):
    key = (nq, nk)
    if key not in _NC_CACHE:
        _NC_CACHE[key] = _build(nq, nk)
    return _NC_CACHE[key]


def kernel(x, kv, mask, attn_bias, WQ_w, WQ_b, WK_w, WK_b, WV_w, WV_b, WO_w, WO_b):
    x = np.asarray(x, dtype=np.float32)
    kv = np.asarray(kv, dtype=np.float32)
    mask = np.asarray(mask)
    attn_bias = np.asarray(attn_bias, dtype=np.float32)
    WQ_w = np.asarray(WQ_w, dtype=np.float32)
    WQ_b = np.asarray(WQ_b, dtype=np.float32)
    WK_w = np.asarray(WK_w, dtype=np.float32)
    WK_b = np.asarray(WK_b, dtype=np.float32)
    WV_w = np.asarray(WV_w, dtype=np.float32)
    WV_b = np.asarray(WV_b, dtype=np.float32)
    WO_w = np.asarray(WO_w, dtype=np.float32)
    WO_b = np.asarray(WO_b, dtype=np.float32)

    sc = 1.0 / math.sqrt(DK)
    maskf = mask.astype(np.float32)

    idxs = [np.nonzero(mask[b])[0] for b in range(B)]
    counts = [len(ix) for ix in idxs]
    nq = nk = max(1, max((c + P - 1) // P for c in counts))
    SQ = SK = nq * P

    def _tile_chunks(aT, n):
        # [E, n*P] -> [n, E//P, P, P]
        return np.ascontiguousarray(
            aT.reshape(NET, P, n, P).transpose(2, 0, 1, 3)
        )

    xTs, kvTs, ebTs = [], [], []
    for b in range(B):
        ix = idxs[b]
        xg = np.zeros((SQ, E), np.float32)
        xg[: counts[b]] = x[b][ix]
        kvg = np.zeros((SK, E), np.float32)
        kvg[: counts[b]] = kv[b][ix]
        ebg = np.zeros((SK, SQ), np.float32)
        ebg[: counts[b], : counts[b]] = np.exp(attn_bias[b][np.ix_(ix, ix)]).T
        # padded query columns: give them one nonzero weight so their
        # denominator is finite (results are discarded by the scatter)
        ebg[0, counts[b] :] = 1.0
        xTs.append(_tile_chunks(xg.T, nq))
        kvTs.append(_tile_chunks(kvg.T, nk))
        # [SK, SQ] -> [nk, P, nq, P]
        ebTs.append(
            np.ascontiguousarray(
                ebg.astype(ml_dtypes.bfloat16)
                .reshape(nk, P, nq, P)
            )
        )

    in_maps = []
    for c in range(NC):
        b, g = c // 4, c % 4
        Dg = slice(DG * g, DG * (g + 1))
        in_maps.append(
            {
                "xT": xTs[b],
                "kvT": kvTs[b],
                "wqT": np.ascontiguousarray((WQ_w[Dg] * sc).T),
                "wkT": np.ascontiguousarray(WK_w[Dg].T),
                "wvT": np.ascontiguousarray(WV_w[Dg].T),
                "bq": np.ascontiguousarray(WQ_b[Dg] * sc),
                "bk": np.ascontiguousarray(WK_b[Dg]),
                "bv": np.ascontiguousarray(WV_b[Dg]),
                "ebT": ebTs[b],
                "r": np.ascontiguousarray(WO_w[:, Dg].T),
                "ones1": np.ones(nk * HPC * (DK + 1), ml_dtypes.bfloat16),
                "onespe": np.ones(DK, np.float32),
            }
        )

    nc = _get_nc(nq, nk)
    res = run_bass_kernel_spmd(nc, in_maps, list(range(NC)), trace=TRACE)
    LAST_RESULTS["res"] = res

    out = np.zeros((B, S, E), np.float32)
    for b in range(B):
        acc = np.zeros((SQ, E), np.float64)
        for g in range(4):
            ot = res.results[b * 4 + g]["out"]  # [nq, 2, P, 512]
            acc += ot.transpose(0, 2, 1, 3).reshape(SQ, E).astype(np.float64)
        acc += WO_b.astype(np.float64)[None, :]
        full = np.zeros((S, E), np.float64)
        full[idxs[b]] = acc[: counts[b]]
        # masked-query rows: reference softmax of an all(-1e9) row is uniform
        mrows = maskf[b] == 0.0
        if mrows.any():
            meanV = (
                kv[b].astype(np.float64).mean(axis=0) @ WV_w.astype(np.float64).T
                + WV_b.astype(np.float64)
            )
            mo = meanV @ WO_w.astype(np.float64).T + WO_b.astype(np.float64)
            full[mrows, :] = mo[None, :]
        out[b] = full.astype(np.float32)
    return out


# revision 15
# speedup vs baseline: 3.0373x; 1.4747x over previous
"""Multi-head attention TRN2 kernel (8 NeuronCores).

Sharding: batch (2) x head-group (4) data/tensor parallel -> 8 cores.
Core c handles batch b = c // 4 and heads [4g, 4g+4) where g = c % 4
(E-dim slice Dg = [256*g, 256*g+256)).

Mask-driven compaction (host side): the reference zeroes attention
weights of masked KEYS exactly (softmax of -1e9 underflows to 0.0 in
f32), and rows for masked QUERIES are recomputed on the host (uniform
attention), so the device only processes gathered unmasked positions:
  q' = count(mask)  padded to a multiple of 128   (queries)
  k' = count(mask)  padded to a multiple of 128   (keys; zero-padded kv
       and zero ebT rows make padding exactly weightless)
For the bench mask (~50% ones) this quarters the attention work and
halves the projections.

Device computes, per core, using transposed layouts throughout:
  QT = (WQ[Dg]/8) @ xg.T          [256, q']   (1/8 = 1/sqrt(DK))
  KT = WK[Dg] @ kvg.T             [256, k']
  V  = kvg @ WV[Dg].T             [k', 256]  (+ ones column per head)
  ST = KT_h.T-blocks @ QT_h       [k, q] scores, transposed
  e  = exp(ST) * ebT              ebT = exp(attn_bias.T) gathered (host)
  U' = [V_h | 1].T @ e            rows 0..63 = unnorm. head out.T, row 64 = denom
  UN = U'[0:64] / denom
  out_partial = UN.T-blocks @ WO[:, Dg].T   [q', 1024]

Attention runs per q-block (<=512 wide) in two head-pair passes so PSUM
holds: 2 rotating score buffers (4 banks) + U' accumulator (2 banks) +
out-proj buffers (2 banks).  AV matmuls trail scores by TRAIL kt-tiles
to hide the exp->mul latency; the U'-normalization epilogue is spread
over the first units of the NEXT pass with its multiplies on the idle
GpSimd engine; the out-projection of the previous q-block interleaves
into the score stream so the PE never drains.

Host: shards/gathers/transposes inputs, folds scale+exp(bias); sums the
4 row-parallel WO partials per batch, adds WO_b, scatters rows back and
overwrites masked-query rows with the uniform-attention value.
"""

import math
from contextlib import ExitStack

import ml_dtypes
import numpy as np

import concourse.bass as bass
import concourse.bacc as bacc
import concourse.tile as tile
from concourse import mybir
from concourse.bass_utils import run_bass_kernel_spmd

F32 = mybir.dt.float32
F32R = mybir.dt.float32r
BF16 = mybir.dt.bfloat16
AF = mybir.ActivationFunctionType

B, S, E, H, DK = 2, 2048, 1024, 16, 64
NC = 8
DG = 256          # dims per core (4 heads x 64)
HPC = 4           # heads per core
P = 128
NET = E // P      # 8 contraction tiles over E
TRAIL = 2         # AV matmuls trail scores by this many kt tiles

TRACE = False
LAST_RESULTS = {}

_NC_CACHE = {}


def _blocks(n):
    """Split n chunks into blocks of 2..4 chunks (a single block may be 1)."""
    out = []
    while n > 4:
        take = 4 if n - 4 != 1 else 3
        out.append(take)
        n -= take
    out.append(n)
    return out


def _build(nq, nk):
    """nq, nk: number of 128-wide query / key chunks."""
    qblocks = _blocks(nq)
    kblocks = _blocks(nk)
    SQ, SK = nq * P, nk * P

    nc = bacc.Bacc("TRN2", target_bir_lowering=False, debug=False, num_devices=NC)
    # chunk-granular tilings: [chunk, et, P, 128]
    xT = nc.dram_tensor("xT", [nq, NET, P, P], F32R, kind="ExternalInput").ap()
    kvT = nc.dram_tensor("kvT", [nk, NET, P, P], F32R, kind="ExternalInput").ap()
    wqT = nc.dram_tensor("wqT", [E, DG], F32R, kind="ExternalInput").ap()
    wkT = nc.dram_tensor("wkT", [E, DG], F32R, kind="ExternalInput").ap()
    wvT = nc.dram_tensor("wvT", [E, DG], F32R, kind="ExternalInput").ap()
    bq = nc.dram_tensor("bq", [DG], F32, kind="ExternalInput")
    bk = nc.dram_tensor("bk", [DG], F32, kind="ExternalInput")
    bv = nc.dram_tensor("bv", [DG], F32, kind="ExternalInput")
    # [ktile, P, qchunk, 128]
    ebT = nc.dram_tensor("ebT", [nk, P, nq, P], BF16, kind="ExternalInput").ap()
    r = nc.dram_tensor("r", [DG, E], F32R, kind="ExternalInput").ap()
    ones1 = nc.dram_tensor("ones1", [nk * HPC * P], BF16, kind="ExternalInput")
    # [qchunk, eb, P, 512]; host reassembles
    out = nc.dram_tensor("out", [nq, 2, P, 512], F32, kind="ExternalOutput").ap()

    wqv = wqT.rearrange("(t p) d -> p t d", p=P)
    wkv = wkT.rearrange("(t p) d -> p t d", p=P)
    wvv = wvT.rearrange("(t p) d -> p t d", p=P)

    with tile.TileContext(nc) as tc, ExitStack() as ctx:
        const = ctx.enter_context(tc.tile_pool(name="const", bufs=1))

        wq_sb = const.tile([P, NET, DG], F32R, name="wq_sb")
        wk_sb = const.tile([P, NET, DG], F32R, name="wk_sb")
        wv_sb = const.tile([P, NET, DG], F32R, name="wv_sb")
        vp_sb = const.tile([P, nk, HPC, P], BF16, name="vp_sb")
        bq_sb = const.tile([P, 2], F32, name="bq_sb")
        bk_sb = const.tile([P, 2], F32, name="bk_sb")
        bvb_sb = const.tile([P, DG], F32, name="bvb_sb")
        r_sb = const.tile([P, 2, E], F32R, name="r_sb")
        qt_sb = const.tile([P, 2, SQ], F32R, name="qt_sb")
        kt_sb = const.tile([P, 2, SK], F32R, name="kt_sb")
        un_sb = const.tile([P, 2, SQ], F32R, name="un_sb")

        # ebT double-buffered per q-block: [P, nk, 512] bf16
        ebpool = ctx.enter_context(tc.tile_pool(name="ebp", bufs=2))

        def emit_ebt_dma(ebt_tile, c0, bq_):
            for kt in range(nk):
                nc.sync.dma_start(
                    out=ebt_tile[:, kt, 0 : bq_ * P],
                    in_=ebT[kt, :, c0 : c0 + bq_].rearrange("p c q -> p (c q)"),
                )

        ebt_tiles = {}
        qstart = [0]
        for bsz in qblocks:
            qstart.append(qstart[-1] + bsz)

        # ---- Phase B: projections ----
        # DMA emission order is latency-critical: first matmuls need only the
        # first weight chunks and the first kv chunk; ebT/r stream later.
        with tc.tile_pool(name="xk", bufs=8) as xkpool, tc.tile_pool(
            name="pj_ps", bufs=1, space="PSUM"
        ) as pj:
            # K/V projections over k-blocks
            c0 = 0
            for bi, bsz in enumerate(kblocks):
                ks = slice(c0 * P, (c0 + bsz) * P)
                ps_k = [pj.tile([P, 512], F32, tag=f"psk{d}", name=f"psk{d}") for d in range(2)]
                ps_v = [pj.tile([P, DG], F32, tag=f"psv{k}", name=f"psv{k}") for k in range(4)]
                kvts = []
                for eg in range(4):
                    es2 = slice(eg * 2, (eg + 1) * 2)
                    if bi == 0:
                        nc.sync.dma_start(out=wk_sb[:, es2], in_=wkv[:, es2])
                        nc.sync.dma_start(out=wv_sb[:, es2], in_=wvv[:, es2])
                    kvt2 = xkpool.tile([P, 2, 512], F32R, tag="kvt", name="kvt2")
                    kvts.append(kvt2)
                    for ei in range(2):
                        nc.sync.dma_start(
                            out=kvt2[:, ei, 0 : bsz * P].rearrange(
                                "p (c q) -> p c q", c=bsz
                            ),
                            in_=kvT[c0 : c0 + bsz, eg * 2 + ei].rearrange(
                                "c p q -> p c q"
                            ),
                        )
                    if bi == 0 and eg == 0:
                        nc.sync.dma_start(
                            out=bq_sb, in_=bq.ap().rearrange("(t p) -> p t", p=P)
                        )
                        nc.sync.dma_start(
                            out=bk_sb, in_=bk.ap().rearrange("(t p) -> p t", p=P)
                        )
                        nc.sync.dma_start(
                            out=bvb_sb,
                            in_=bass.AP(tensor=bv, offset=0, ap=[[0, P], [1, DG]]),
                        )
                        nc.sync.dma_start(
                            out=vp_sb.rearrange("p a b c -> p (a b c)"),
                            in_=bass.AP(
                                tensor=ones1, offset=0,
                                ap=[[0, P], [1, nk * HPC * P]],
                            ),
                        )
                for eg in range(4):
                    kvt2 = kvts[eg]
                    for ei in range(2):
                        et = eg * 2 + ei
                        kvt = kvt2[:, ei, 0 : bsz * P]
                        st, sp = (et == 0), (et == NET - 1)
                        for d in range(2):
                            nc.tensor.matmul(
                                ps_k[d][:, 0 : bsz * P],
                                wk_sb[:, et, d * P : (d + 1) * P], kvt,
                                start=st, stop=sp,
                            )
                        for kb in range(bsz):
                            nc.tensor.matmul(
                                ps_v[kb], kvt[:, kb * P : (kb + 1) * P],
                                wv_sb[:, et, :], start=st, stop=sp,
                            )
                for d in range(2):
                    nc.vector.tensor_scalar_add(
                        kt_sb[:, d, ks], ps_k[d][:, 0 : bsz * P], bk_sb[:, d : d + 1]
                    )
                for kb in range(bsz):
                    nc.vector.tensor_add(
                        vp_sb[:, c0 + kb, :, DK : 2 * DK],
                        ps_v[kb].rearrange("p (h d) -> p h d", h=HPC),
                        bvb_sb.rearrange("p (h d) -> p h d", h=HPC),
                    )
                if bi == 0:
                    ebt_tiles[0] = ebpool.tile([P, nk, 512], BF16, tag="ebt", name="ebt0")
                    emit_ebt_dma(ebt_tiles[0], 0, qblocks[0])
                    nc.sync.dma_start(
                        out=r_sb, in_=r.rearrange("(t p) e -> p t e", p=P)
                    )
                c0 += bsz
            # Q projections over q-blocks
            c0 = 0
            for bi, bsz in enumerate(qblocks):
                qs = slice(c0 * P, (c0 + bsz) * P)
                ps_q = [pj.tile([P, 512], F32, tag=f"psq{d}", name=f"psq{d}") for d in range(2)]
                xts = []
                for eg in range(4):
                    es2 = slice(eg * 2, (eg + 1) * 2)
                    if bi == 0:
                        nc.sync.dma_start(out=wq_sb[:, es2], in_=wqv[:, es2])
                    xt2 = xkpool.tile([P, 2, 512], F32R, tag="xt", name="xt2")
                    xts.append(xt2)
                    for ei in range(2):
                        nc.sync.dma_start(
                            out=xt2[:, ei, 0 : bsz * P].rearrange(
                                "p (c q) -> p c q", c=bsz
                            ),
                            in_=xT[c0 : c0 + bsz, eg * 2 + ei].rearrange(
                                "c p q -> p c q"
                            ),
                        )
                for eg in range(4):
                    xt2 = xts[eg]
                    for ei in range(2):
                        et = eg * 2 + ei
                        xt = xt2[:, ei, 0 : bsz * P]
                        st, sp = (et == 0), (et == NET - 1)
                        for d in range(2):
                            nc.tensor.matmul(
                                ps_q[d][:, 0 : bsz * P],
                                wq_sb[:, et, d * P : (d + 1) * P], xt,
                                start=st, stop=sp,
                            )
                for d in range(2):
                    nc.vector.tensor_scalar_add(
                        qt_sb[:, d, qs], ps_q[d][:, 0 : bsz * P], bq_sb[:, d : d + 1]
                    )
                c0 += bsz

        # ---- Phase C: attention + interleaved out-projection ----
        with tc.tile_pool(name="fp", bufs=3) as fpool, tc.tile_pool(
            name="ep", bufs=TRAIL + 3
        ) as epool, tc.tile_pool(name="dn", bufs=2) as dpool, tc.tile_pool(
            name="osb", bufs=3
        ) as opool, tc.tile_pool(
            name="s_ps", bufs=2, space="PSUM"
        ) as sps, tc.tile_pool(
            name="u_ps", bufs=1, space="PSUM"
        ) as ups, tc.tile_pool(name="o_ps", bufs=2, space="PSUM") as ops:

            def d_unit(qchunk, eb):
                """Out-projection for one (128-query chunk, 512-col half)."""
                rs = slice(qchunk * P, (qchunk + 1) * P)
                es = slice(eb * 512, (eb + 1) * 512)
                ps_o = ops.tile([P, 512], F32, tag="pso", name="pso")
                for d in range(2):
                    nc.tensor.matmul(
                        ps_o, un_sb[:, d, rs], r_sb[:, d, es],
                        start=(d == 0), stop=(d == 1),
                    )
                osb = opool.tile([P, 512], F32, tag="osb", name="osb")
                if eb == 0:
                    nc.scalar.activation(osb, ps_o, AF.Copy)
                else:
                    nc.vector.tensor_copy(osb, ps_o)
                nc.sync.dma_start(out=out[qchunk, eb], in_=osb)

            def epi_step(st, step):
                """One piece of a finished pass's deferred epilogue, spread
                over the next pass so no engine queue blocks for long.  V'
                carries 64 replicated ones-columns, so U' rows 64..127 hold
                64 copies of the denominator: the reciprocal runs partition-
                parallel and the normalizing multiply needs no broadcast."""
                w, hp, qs, u = st["w"], st["hp"], st["qs"], st["u"]
                if step == 0:
                    rd = dpool.tile([DK, 2, 512], F32, tag="rd", name="rd")
                    nc.vector.reciprocal_approx_fast(
                        rd[:, :, 0:w], u[0:DK, :, 0:w]
                    )
                    st["rd"] = rd
                    un = dpool.tile([DK, 2, 512], F32, tag="unum", name="unum")
                    nc.sync.dma_start(
                        out=un[:, :, 0:w], in_=u[DK : 2 * DK, :, 0:w]
                    )
                    st["un"] = un
                elif step in (1, 2):
                    j = step - 1
                    nc.vector.tensor_mul(
                        un_sb[j * DK : (j + 1) * DK, hp, qs],
                        st["un"][:, j, 0:w], st["rd"][:, j, 0:w],
                    )

            epi_pend = None  # (state dict, emitted-steps)
            d_pend = []      # (qchunk, eb) out-proj units awaiting a slot

            for qi, bsz in enumerate(qblocks):
                c0 = qstart[qi]
                w = bsz * P
                qs = slice(c0 * P, (c0 + bsz) * P)
                ebt = ebt_tiles.pop(qi)
                for hp in range(2):  # head pair: heads {2*hp, 2*hp+1}
                    ps_u = ups.tile([2 * DK, 2, 512], F32, tag="psu", name="psu")
                    pend = []
                    for kt2 in range(nk):
                        if epi_pend is not None and kt2 in (0, 2, 3):
                            epi_step(epi_pend, 0 if kt2 == 0 else kt2 - 1)
                            if kt2 == 3:
                                epi_pend = None
                        if d_pend and kt2 >= 4:
                            d_unit(*d_pend.pop(0))
                        if hp == 0 and kt2 == 2 and qi + 1 < len(qblocks):
                            ebt_tiles[qi + 1] = ebpool.tile(
                                [P, nk, 512], BF16, tag="ebt", name="ebt1"
                            )
                            emit_ebt_dma(ebt_tiles[qi + 1], qstart[qi + 1], qblocks[qi + 1])
                        ks = slice(kt2 * P, (kt2 + 1) * P)
                        ps_s = sps.tile([P, 2, 512], F32, tag="pss", name="pss")
                        for j in range(2):
                            po = j * DK
                            nc.tensor.matmul(
                                ps_s[:, j, 0:w], kt_sb[po : po + DK, hp, ks],
                                qt_sb[po : po + DK, hp, qs], start=True, stop=True,
                            )
                        f2 = fpool.tile([P, 2, 512], BF16, tag="f", name="f2")
                        nc.scalar.activation(
                            f2[:, :, 0:w], ps_s[:, :, 0:w], AF.Exp
                        )
                        e2 = epool.tile([P, 2, 512], BF16, tag="e", name="e2")
                        for j in range(2):
                            nc.vector.tensor_mul(
                                e2[:, j, 0:w], f2[:, j, 0:w], ebt[:, kt2, 0:w]
                            )
                        pend.append((kt2, e2))
                        if len(pend) > TRAIL:
                            pkt, pe2 = pend.pop(0)
                            for j in range(2):
                                nc.tensor.matmul(
                                    ps_u[:, j, 0:w], vp_sb[:, pkt, 2 * hp + j, :],
                                    pe2[:, j, 0:w], start=(pkt == 0), stop=False,
                                )
                    for idx, (pkt, pe2) in enumerate(pend):
                        last = idx == len(pend) - 1
                        for j in range(2):
                            nc.tensor.matmul(
                                ps_u[:, j, 0:w], vp_sb[:, pkt, 2 * hp + j, :],
                                pe2[:, j, 0:w], start=(pkt == 0), stop=last,
                            )
                    # evict U' now (frees the PSUM accumulator); the rest of
                    # the epilogue is spread over the next pass's units.
                    u_raw = dpool.tile([2 * DK, 2, 512], F32, tag="uraw", name="u_raw")
                    nc.scalar.activation(u_raw[:, :, 0:w], ps_u[:, :, 0:w], AF.Copy)
                    if epi_pend is not None:  # nk < 4: flush leftovers
                        for step in range(3):
                            epi_step(epi_pend, step)
                    epi_pend = {"u": u_raw, "hp": hp, "qs": qs, "w": w}
                    if hp == 1:
                        d_pend.extend((c0 + c, eb) for c in range(bsz) for eb in range(2))
            # tail: last pass's epilogue + remaining out-proj units
            for step in range(3):
                epi_step(epi_pend, step)
            for unit in d_pend:
                d_unit(*unit)

    nc.compile()
    return nc


def _get_nc(nq, nk):
    key = (nq, nk)
    if key not in _NC_CACHE:
        _NC_CACHE[key] = _build(nq, nk)
    return _NC_CACHE[key]


def kernel(x, kv, mask, attn_bias, WQ_w, WQ_b, WK_w, WK_b, WV_w, WV_b, WO_w, WO_b):
    x = np.asarray(x, dtype=np.float32)
    kv = np.asarray(kv, dtype=np.float32)
    mask = np.asarray(mask)
    attn_bias = np.asarray(attn_bias, dtype=np.float32)
    WQ_w = np.asarray(WQ_w, dtype=np.float32)
    WQ_b = np.asarray(WQ_b, dtype=np.float32)
    WK_w = np.asarray(WK_w, dtype=np.float32)
    WK_b = np.asarray(WK_b, dtype=np.float32)
    WV_w = np.asarray(WV_w, dtype=np.float32)
    WV_b = np.asarray(WV_b, dtype=np.float32)
    WO_w = np.asarray(WO_w, dtype=np.float32)
    WO_b = np.asarray(WO_b, dtype=np.float32)

    sc = 1.0 / math.sqrt(DK)
    maskf = mask.astype(np.float32)

    idxs = [np.nonzero(mask[b])[0] for b in range(B)]
    counts = [len(ix) for ix in idxs]
    nq = nk = max(1, max((c + P - 1) // P for c in counts))
    SQ = SK = nq * P

    def _tile_chunks(aT, n):
        # [E, n*P] -> [n, E//P, P, P]
        return np.ascontiguousarray(
            aT.reshape(NET, P, n, P).transpose(2, 0, 1, 3)
        )

    xTs, kvTs, ebTs = [], [], []
    for b in range(B):
        ix = idxs[b]
        xg = np.zeros((SQ, E), np.float32)
        xg[: counts[b]] = x[b][ix]
        kvg = np.zeros((SK, E), np.float32)
        kvg[: counts[b]] = kv[b][ix]
        ebg = np.zeros((SK, SQ), np.float32)
        ebg[: counts[b], : counts[b]] = np.exp(attn_bias[b][np.ix_(ix, ix)]).T
        # padded query columns: give them one nonzero weight so their
        # denominator is finite (results are discarded by the scatter)
        ebg[0, counts[b] :] = 1.0
        xTs.append(_tile_chunks(xg.T, nq))
        kvTs.append(_tile_chunks(kvg.T, nk))
        # [SK, SQ] -> [nk, P, nq, P]
        ebTs.append(
            np.ascontiguousarray(
                ebg.astype(ml_dtypes.bfloat16)
                .reshape(nk, P, nq, P)
            )
        )

    in_maps = []
    for c in range(NC):
        b, g = c // 4, c % 4
        Dg = slice(DG * g, DG * (g + 1))
        in_maps.append(
            {
                "xT": xTs[b],
                "kvT": kvTs[b],
                "wqT": np.ascontiguousarray((WQ_w[Dg] * sc).T),
                "wkT": np.ascontiguousarray(WK_w[Dg].T),
                "wvT": np.ascontiguousarray(WV_w[Dg].T),
                "bq": np.ascontiguousarray(WQ_b[Dg] * sc),
                "bk": np.ascontiguousarray(WK_b[Dg]),
                "bv": np.ascontiguousarray(WV_b[Dg]),
                "ebT": ebTs[b],
                "r": np.ascontiguousarray(WO_w[:, Dg].T),
                "ones1": np.ones(nk * HPC * P, ml_dtypes.bfloat16),
            }
        )

    nc = _get_nc(nq, nk)
    res = run_bass_kernel_spmd(nc, in_maps, list(range(NC)), trace=TRACE)
    LAST_RESULTS["res"] = res

    out = np.zeros((B, S, E), np.float32)
    for b in range(B):
        acc = np.zeros((SQ, E), np.float64)
        for g in range(4):
            ot = res.results[b * 4 + g]["out"]  # [nq, 2, P, 512]
            acc += ot.transpose(0, 2, 1, 3).reshape(SQ, E).astype(np.float64)
        acc += WO_b.astype(np.float64)[None, :]
        full = np.zeros((S, E), np.float64)
        full[idxs[b]] = acc[: counts[b]]
        # masked-query rows: reference softmax of an all(-1e9) row is uniform
        mrows = maskf[b] == 0.0
        if mrows.any():
            meanV = (
                kv[b].astype(np.float64).mean(axis=0) @ WV_w.astype(np.float64).T
                + WV_b.astype(np.float64)
            )
            mo = meanV @ WO_w.astype(np.float64).T + WO_b.astype(np.float64)
            full[mrows, :] = mo[None, :]
        out[b] = full.astype(np.float32)
    return out


# revision 16
# speedup vs baseline: 3.2336x; 1.0646x over previous
"""Multi-head attention TRN2 kernel (8 NeuronCores).

Sharding: batch (2) x head-group (4) data/tensor parallel -> 8 cores.
Core c handles batch b = c // 4 and heads [4g, 4g+4) where g = c % 4
(E-dim slice Dg = [256*g, 256*g+256)).

Mask-driven compaction (host side): the reference zeroes attention
weights of masked KEYS exactly (softmax of -1e9 underflows to 0.0 in
f32), and rows for masked QUERIES are recomputed on the host (uniform
attention), so the device only processes gathered unmasked positions:
  q' = count(mask)  padded to a multiple of 128   (queries)
  k' = count(mask)  padded to a multiple of 128   (keys; zero-padded kv
       and zero ebT rows make padding exactly weightless)
For the bench mask (~50% ones) this quarters the attention work and
halves the projections.

Device computes, per core, using transposed layouts throughout:
  QT = (WQ[Dg]/8) @ xg.T          [256, q']   (1/8 = 1/sqrt(DK))
  KT = WK[Dg] @ kvg.T             [256, k']
  V  = kvg @ WV[Dg].T             [k', 256]  (+ ones column per head)
  ST = KT_h.T-blocks @ QT_h       [k, q] scores, transposed
  e  = exp(ST) * ebT              ebT = exp(attn_bias.T) gathered (host)
  U' = [V_h | 1].T @ e            rows 0..63 = unnorm. head out.T, row 64 = denom
  UN = U'[0:64] / denom
  out_partial = UN.T-blocks @ WO[:, Dg].T   [q', 1024]

Attention runs per q-block (<=512 wide) in two head-pair passes so PSUM
holds: 2 rotating score buffers (4 banks) + U' accumulator (2 banks) +
out-proj buffers (2 banks).  AV matmuls trail scores by TRAIL kt-tiles
to hide the exp->mul latency; the U'-normalization epilogue is spread
over the first units of the NEXT pass with its multiplies on the idle
GpSimd engine; the out-projection of the previous q-block interleaves
into the score stream so the PE never drains.

Host: shards/gathers/transposes inputs, folds scale+exp(bias); sums the
4 row-parallel WO partials per batch, adds WO_b, scatters rows back and
overwrites masked-query rows with the uniform-attention value.
"""

import math
from contextlib import ExitStack

import ml_dtypes
import numpy as np

import concourse.bass as bass
import concourse.bacc as bacc
import concourse.tile as tile
from concourse import mybir
from concourse.bass_utils import run_bass_kernel_spmd

F32 = mybir.dt.float32
F32R = mybir.dt.float32r
BF16 = mybir.dt.bfloat16
AF = mybir.ActivationFunctionType

B, S, E, H, DK = 2, 2048, 1024, 16, 64
NC = 8
DG = 256          # dims per core (4 heads x 64)
HPC = 4           # heads per core
P = 128
NET = E // P      # 8 contraction tiles over E
TRAIL = 2         # AV matmuls trail scores by this many kt tiles

TRACE = False
LAST_RESULTS = {}

_NC_CACHE = {}


def _blocks(n):
    """Split n chunks into blocks of 2..4 chunks (a single block may be 1)."""
    out = []
    while n > 4:
        take = 4 if n - 4 != 1 else 3
        out.append(take)
        n -= take
    out.append(n)
    return out


def _build(nq, nk):
    """nq, nk: number of 128-wide query / key chunks."""
    qblocks = _blocks(nq)
    kblocks = _blocks(nk)
    SQ, SK = nq * P, nk * P

    nc = bacc.Bacc("TRN2", target_bir_lowering=False, debug=False, num_devices=NC)
    # chunk-granular tilings: [chunk, et, P, 128]
    xT = nc.dram_tensor("xT", [nq, NET, P, P], BF16, kind="ExternalInput").ap()
    kvT = nc.dram_tensor("kvT", [nk, NET, P, P], BF16, kind="ExternalInput").ap()
    wqT = nc.dram_tensor("wqT", [E, DG], BF16, kind="ExternalInput").ap()
    wkT = nc.dram_tensor("wkT", [E, DG], BF16, kind="ExternalInput").ap()
    wvT = nc.dram_tensor("wvT", [E, DG], BF16, kind="ExternalInput").ap()
    bq = nc.dram_tensor("bq", [DG], F32, kind="ExternalInput")
    bk = nc.dram_tensor("bk", [DG], F32, kind="ExternalInput")
    bv = nc.dram_tensor("bv", [DG], F32, kind="ExternalInput")
    # [ktile, P, qchunk, 128]
    ebT = nc.dram_tensor("ebT", [nk, P, nq, P], BF16, kind="ExternalInput").ap()
    r = nc.dram_tensor("r", [DG, E], BF16, kind="ExternalInput").ap()
    ones1 = nc.dram_tensor("ones1", [nk * HPC * P], BF16, kind="ExternalInput")
    # [qchunk, eb, P, 512]; host reassembles
    out = nc.dram_tensor("out", [nq, 2, P, 512], F32, kind="ExternalOutput").ap()

    wqv = wqT.rearrange("(t p) d -> p t d", p=P)
    wkv = wkT.rearrange("(t p) d -> p t d", p=P)
    wvv = wvT.rearrange("(t p) d -> p t d", p=P)

    with tile.TileContext(nc) as tc, ExitStack() as ctx:
        const = ctx.enter_context(tc.tile_pool(name="const", bufs=1))

        wq_sb = const.tile([P, NET, DG], BF16, name="wq_sb")
        wk_sb = const.tile([P, NET, DG], BF16, name="wk_sb")
        wv_sb = const.tile([P, NET, DG], BF16, name="wv_sb")
        vp_sb = const.tile([P, nk, HPC, P], BF16, name="vp_sb")
        bq_sb = const.tile([P, 2], F32, name="bq_sb")
        bk_sb = const.tile([P, 2], F32, name="bk_sb")
        bvb_sb = const.tile([P, DG], F32, name="bvb_sb")
        r_sb = const.tile([P, 2, E], BF16, name="r_sb")
        qt_sb = const.tile([P, 2, SQ], BF16, name="qt_sb")
        kt_sb = const.tile([P, 2, SK], BF16, name="kt_sb")
        un_sb = const.tile([P, 2, SQ], BF16, name="un_sb")

        # ebT double-buffered per q-block: [P, nk, 512] bf16
        ebpool = ctx.enter_context(tc.tile_pool(name="ebp", bufs=2))

        def emit_ebt_dma(ebt_tile, c0, bq_):
            for kt in range(nk):
                nc.sync.dma_start(
                    out=ebt_tile[:, kt, 0 : bq_ * P],
                    in_=ebT[kt, :, c0 : c0 + bq_].rearrange("p c q -> p (c q)"),
                )

        ebt_tiles = {}
        qstart = [0]
        for bsz in qblocks:
            qstart.append(qstart[-1] + bsz)

        # ---- Phase B: projections ----
        # DMA emission order is latency-critical: first matmuls need only the
        # first weight chunks and the first kv chunk; ebT/r stream later.
        with tc.tile_pool(name="xk", bufs=8) as xkpool, tc.tile_pool(
            name="pj_ps", bufs=1, space="PSUM"
        ) as pj:
            # K/V projections over k-blocks
            c0 = 0
            for bi, bsz in enumerate(kblocks):
                ks = slice(c0 * P, (c0 + bsz) * P)
                ps_k = [pj.tile([P, 512], F32, tag=f"psk{d}", name=f"psk{d}") for d in range(2)]
                ps_v = [pj.tile([P, DG], F32, tag=f"psv{k}", name=f"psv{k}") for k in range(4)]
                kvts = []
                for eg in range(4):
                    es2 = slice(eg * 2, (eg + 1) * 2)
                    if bi == 0:
                        nc.sync.dma_start(out=wk_sb[:, es2], in_=wkv[:, es2])
                        nc.sync.dma_start(out=wv_sb[:, es2], in_=wvv[:, es2])
                    kvt2 = xkpool.tile([P, 2, 512], BF16, tag="kvt", name="kvt2")
                    kvts.append(kvt2)
                    for ei in range(2):
                        nc.sync.dma_start(
                            out=kvt2[:, ei, 0 : bsz * P].rearrange(
                                "p (c q) -> p c q", c=bsz
                            ),
                            in_=kvT[c0 : c0 + bsz, eg * 2 + ei].rearrange(
                                "c p q -> p c q"
                            ),
                        )
                    if bi == 0 and eg == 0:
                        nc.sync.dma_start(
                            out=bq_sb, in_=bq.ap().rearrange("(t p) -> p t", p=P)
                        )
                        nc.sync.dma_start(
                            out=bk_sb, in_=bk.ap().rearrange("(t p) -> p t", p=P)
                        )
                        nc.sync.dma_start(
                            out=bvb_sb,
                            in_=bass.AP(tensor=bv, offset=0, ap=[[0, P], [1, DG]]),
                        )
                        nc.sync.dma_start(
                            out=vp_sb.rearrange("p a b c -> p (a b c)"),
                            in_=bass.AP(
                                tensor=ones1, offset=0,
                                ap=[[0, P], [1, nk * HPC * P]],
                            ),
                        )
                for eg in range(4):
                    kvt2 = kvts[eg]
                    for ei in range(2):
                        et = eg * 2 + ei
                        kvt = kvt2[:, ei, 0 : bsz * P]
                        st, sp = (et == 0), (et == NET - 1)
                        for d in range(2):
                            nc.tensor.matmul(
                                ps_k[d][:, 0 : bsz * P],
                                wk_sb[:, et, d * P : (d + 1) * P], kvt,
                                start=st, stop=sp,
                            )
                        for kb in range(bsz):
                            nc.tensor.matmul(
                                ps_v[kb], kvt[:, kb * P : (kb + 1) * P],
                                wv_sb[:, et, :], start=st, stop=sp,
                            )
                for d in range(2):
                    nc.vector.tensor_scalar_add(
                        kt_sb[:, d, ks], ps_k[d][:, 0 : bsz * P], bk_sb[:, d : d + 1]
                    )
                for kb in range(bsz):
                    nc.vector.tensor_add(
                        vp_sb[:, c0 + kb, :, DK : 2 * DK],
                        ps_v[kb].rearrange("p (h d) -> p h d", h=HPC),
                        bvb_sb.rearrange("p (h d) -> p h d", h=HPC),
                    )
                if bi == 0:
                    ebt_tiles[0] = ebpool.tile([P, nk, 512], BF16, tag="ebt", name="ebt0")
                    emit_ebt_dma(ebt_tiles[0], 0, qblocks[0])
                    nc.sync.dma_start(
                        out=r_sb, in_=r.rearrange("(t p) e -> p t e", p=P)
                    )
                c0 += bsz
            # Q projections over q-blocks
            c0 = 0
            for bi, bsz in enumerate(qblocks):
                qs = slice(c0 * P, (c0 + bsz) * P)
                ps_q = [pj.tile([P, 512], F32, tag=f"psq{d}", name=f"psq{d}") for d in range(2)]
                xts = []
                for eg in range(4):
                    es2 = slice(eg * 2, (eg + 1) * 2)
                    if bi == 0:
                        nc.sync.dma_start(out=wq_sb[:, es2], in_=wqv[:, es2])
                    xt2 = xkpool.tile([P, 2, 512], BF16, tag="xt", name="xt2")
                    xts.append(xt2)
                    for ei in range(2):
                        nc.sync.dma_start(
                            out=xt2[:, ei, 0 : bsz * P].rearrange(
                                "p (c q) -> p c q", c=bsz
                            ),
                            in_=xT[c0 : c0 + bsz, eg * 2 + ei].rearrange(
                                "c p q -> p c q"
                            ),
                        )
                for eg in range(4):
                    xt2 = xts[eg]
                    for ei in range(2):
                        et = eg * 2 + ei
                        xt = xt2[:, ei, 0 : bsz * P]
                        st, sp = (et == 0), (et == NET - 1)
                        for d in range(2):
                            nc.tensor.matmul(
                                ps_q[d][:, 0 : bsz * P],
                                wq_sb[:, et, d * P : (d + 1) * P], xt,
                                start=st, stop=sp,
                            )
                for d in range(2):
                    nc.vector.tensor_scalar_add(
                        qt_sb[:, d, qs], ps_q[d][:, 0 : bsz * P], bq_sb[:, d : d + 1]
                    )
                c0 += bsz

        # ---- Phase C: attention + interleaved out-projection ----
        with tc.tile_pool(name="fp", bufs=3) as fpool, tc.tile_pool(
            name="ep", bufs=TRAIL + 3
        ) as epool, tc.tile_pool(name="dn", bufs=2) as dpool, tc.tile_pool(
            name="osb", bufs=3
        ) as opool, tc.tile_pool(
            name="s_ps", bufs=2, space="PSUM"
        ) as sps, tc.tile_pool(
            name="u_ps", bufs=1, space="PSUM"
        ) as ups, tc.tile_pool(name="o_ps", bufs=2, space="PSUM") as ops:

            def d_unit(qchunk, eb):
                """Out-projection for one (128-query chunk, 512-col half)."""
                rs = slice(qchunk * P, (qchunk + 1) * P)
                es = slice(eb * 512, (eb + 1) * 512)
                ps_o = ops.tile([P, 512], F32, tag="pso", name="pso")
                for d in range(2):
                    nc.tensor.matmul(
                        ps_o, un_sb[:, d, rs], r_sb[:, d, es],
                        start=(d == 0), stop=(d == 1),
                    )
                osb = opool.tile([P, 512], F32, tag="osb", name="osb")
                if eb == 0:
                    nc.scalar.activation(osb, ps_o, AF.Copy)
                else:
                    nc.vector.tensor_copy(osb, ps_o)
                nc.sync.dma_start(out=out[qchunk, eb], in_=osb)

            def epi_step(st, step):
                """One piece of a finished pass's deferred epilogue, spread
                over the next pass so no engine queue blocks for long.  V'
                carries 64 replicated ones-columns, so U' rows 64..127 hold
                64 copies of the denominator: the reciprocal runs partition-
                parallel and the normalizing multiply needs no broadcast."""
                w, hp, qs, u = st["w"], st["hp"], st["qs"], st["u"]
                if step == 0:
                    rd = dpool.tile([DK, 2, 512], F32, tag="rd", name="rd")
                    nc.vector.reciprocal_approx_fast(
                        rd[:, :, 0:w], u[0:DK, :, 0:w]
                    )
                    st["rd"] = rd
                    un = dpool.tile([DK, 2, 512], F32, tag="unum", name="unum")
                    nc.sync.dma_start(
                        out=un[:, :, 0:w], in_=u[DK : 2 * DK, :, 0:w]
                    )
                    st["un"] = un
                elif step in (1, 2):
                    j = step - 1
                    nc.vector.tensor_mul(
                        un_sb[j * DK : (j + 1) * DK, hp, qs],
                        st["un"][:, j, 0:w], st["rd"][:, j, 0:w],
                    )

            epi_pend = None  # (state dict, emitted-steps)
            d_pend = []      # (qchunk, eb) out-proj units awaiting a slot

            for qi, bsz in enumerate(qblocks):
                c0 = qstart[qi]
                w = bsz * P
                qs = slice(c0 * P, (c0 + bsz) * P)
                ebt = ebt_tiles.pop(qi)
                for hp in range(2):  # head pair: heads {2*hp, 2*hp+1}
                    ps_u = ups.tile([2 * DK, 2, 512], F32, tag="psu", name="psu")
                    pend = []
                    for kt2 in range(nk):
                        if epi_pend is not None and kt2 in (0, 2, 3):
                            epi_step(epi_pend, 0 if kt2 == 0 else kt2 - 1)
                            if kt2 == 3:
                                epi_pend = None
                        if d_pend and kt2 >= 4:
                            d_unit(*d_pend.pop(0))
                        if hp == 0 and kt2 == 2 and qi + 1 < len(qblocks):
                            ebt_tiles[qi + 1] = ebpool.tile(
                                [P, nk, 512], BF16, tag="ebt", name="ebt1"
                            )
                            emit_ebt_dma(ebt_tiles[qi + 1], qstart[qi + 1], qblocks[qi + 1])
                        ks = slice(kt2 * P, (kt2 + 1) * P)
                        ps_s = sps.tile([P, 2, 512], F32, tag="pss", name="pss")
                        for j in range(2):
                            po = j * DK
                            nc.tensor.matmul(
                                ps_s[:, j, 0:w], kt_sb[po : po + DK, hp, ks],
                                qt_sb[po : po + DK, hp, qs], start=True, stop=True,
                            )
                        f2 = fpool.tile([P, 2, 512], BF16, tag="f", name="f2")
                        nc.scalar.activation(
                            f2[:, :, 0:w], ps_s[:, :, 0:w], AF.Exp
                        )
                        e2 = epool.tile([P, 2, 512], BF16, tag="e", name="e2")
                        for j in range(2):
                            nc.vector.tensor_mul(
                                e2[:, j, 0:w], f2[:, j, 0:w], ebt[:, kt2, 0:w]
                            )
                        pend.append((kt2, e2))
                        if len(pend) > TRAIL:
                            pkt, pe2 = pend.pop(0)
                            for j in range(2):
                                nc.tensor.matmul(
                                    ps_u[:, j, 0:w], vp_sb[:, pkt, 2 * hp + j, :],
                                    pe2[:, j, 0:w], start=(pkt == 0), stop=False,
                                )
                    for idx, (pkt, pe2) in enumerate(pend):
                        last = idx == len(pend) - 1
                        for j in range(2):
                            nc.tensor.matmul(
                                ps_u[:, j, 0:w], vp_sb[:, pkt, 2 * hp + j, :],
                                pe2[:, j, 0:w], start=(pkt == 0), stop=last,
                            )
                    # evict U' now (frees the PSUM accumulator); the rest of
                    # the epilogue is spread over the next pass's units.
                    u_raw = dpool.tile([2 * DK, 2, 512], F32, tag="uraw", name="u_raw")
                    nc.scalar.activation(u_raw[:, :, 0:w], ps_u[:, :, 0:w], AF.Copy)
                    if epi_pend is not None:  # nk < 4: flush leftovers
                        for step in range(3):
                            epi_step(epi_pend, step)
                    epi_pend = {"u": u_raw, "hp": hp, "qs": qs, "w": w}
                    if hp == 1:
                        d_pend.extend((c0 + c, eb) for c in range(bsz) for eb in range(2))
            # tail: last pass's epilogue + remaining out-proj units
            for step in range(3):
                epi_step(epi_pend, step)
            for unit in d_pend:
                d_unit(*unit)

    nc.compile()
    return nc


def _get_nc(nq, nk):
    key = (nq, nk)
    if key not in _NC_CACHE:
        _NC_CACHE[key] = _build(nq, nk)
    return _NC_CACHE[key]


def kernel(x, kv, mask, attn_bias, WQ_w, WQ_b, WK_w, WK_b, WV_w, WV_b, WO_w, WO_b):
    x = np.asarray(x, dtype=np.float32)
    kv = np.asarray(kv, dtype=np.float32)
    mask = np.asarray(mask)
    attn_bias = np.asarray(attn_bias, dtype=np.float32)
    WQ_w = np.asarray(WQ_w, dtype=np.float32)
    WQ_b = np.asarray(WQ_b, dtype=np.float32)
    WK_w = np.asarray(WK_w, dtype=np.float32)
    WK_b = np.asarray(WK_b, dtype=np.float32)
    WV_w = np.asarray(WV_w, dtype=np.float32)
    WV_b = np.asarray(WV_b, dtype=np.float32)
    WO_w = np.asarray(WO_w, dtype=np.float32)
    WO_b = np.asarray(WO_b, dtype=np.float32)

    sc = 1.0 / math.sqrt(DK)
    maskf = mask.astype(np.float32)

    idxs = [np.nonzero(mask[b])[0] for b in range(B)]
    counts = [len(ix) for ix in idxs]
    nq = nk = max(1, max((c + P - 1) // P for c in counts))
    SQ = SK = nq * P

    def _tile_chunks(aT, n):
        # [E, n*P] -> [n, E//P, P, P]
        return np.ascontiguousarray(
            aT.reshape(NET, P, n, P).transpose(2, 0, 1, 3)
        )

    xTs, kvTs, ebTs = [], [], []
    for b in range(B):
        ix = idxs[b]
        xg = np.zeros((SQ, E), np.float32)
        xg[: counts[b]] = x[b][ix]
        kvg = np.zeros((SK, E), np.float32)
        kvg[: counts[b]] = kv[b][ix]
        ebg = np.zeros((SK, SQ), np.float32)
        ebg[: counts[b], : counts[b]] = np.exp(attn_bias[b][np.ix_(ix, ix)]).T
        # padded query columns: give them one nonzero weight so their
        # denominator is finite (results are discarded by the scatter)
        ebg[0, counts[b] :] = 1.0
        xTs.append(_tile_chunks(xg.T.astype(ml_dtypes.bfloat16), nq))
        kvTs.append(_tile_chunks(kvg.T.astype(ml_dtypes.bfloat16), nk))
        # [SK, SQ] -> [nk, P, nq, P]
        ebTs.append(
            np.ascontiguousarray(
                ebg.astype(ml_dtypes.bfloat16)
                .reshape(nk, P, nq, P)
            )
        )

    in_maps = []
    for c in range(NC):
        b, g = c // 4, c % 4
        Dg = slice(DG * g, DG * (g + 1))
        in_maps.append(
            {
                "xT": xTs[b],
                "kvT": kvTs[b],
                "wqT": np.ascontiguousarray((WQ_w[Dg] * sc).T.astype(ml_dtypes.bfloat16)),
                "wkT": np.ascontiguousarray(WK_w[Dg].T.astype(ml_dtypes.bfloat16)),
                "wvT": np.ascontiguousarray(WV_w[Dg].T.astype(ml_dtypes.bfloat16)),
                "bq": np.ascontiguousarray(WQ_b[Dg] * sc),
                "bk": np.ascontiguousarray(WK_b[Dg]),
                "bv": np.ascontiguousarray(WV_b[Dg]),
                "ebT": ebTs[b],
                "r": np.ascontiguousarray(WO_w[:, Dg].T.astype(ml_dtypes.bfloat16)),
                "ones1": np.ones(nk * HPC * P, ml_dtypes.bfloat16),
            }
        )

    nc = _get_nc(nq, nk)
    res = run_bass_kernel_spmd(nc, in_maps, list(range(NC)), trace=TRACE)
    LAST_RESULTS["res"] = res

    out = np.zeros((B, S, E), np.float32)
    for b in range(B):
        acc = np.zeros((SQ, E), np.float64)
        for g in range(4):
            ot = res.results[b * 4 + g]["out"]  # [nq, 2, P, 512]
            acc += ot.transpose(0, 2, 1, 3).reshape(SQ, E).astype(np.float64)
        acc += WO_b.astype(np.float64)[None, :]
        full = np.zeros((S, E), np.float64)
        full[idxs[b]] = acc[: counts[b]]
        # masked-query rows: reference softmax of an all(-1e9) row is uniform
        mrows = maskf[b] == 0.0
        if mrows.any():
            meanV = (
                kv[b].astype(np.float64).mean(axis=0) @ WV_w.astype(np.float64).T
                + WV_b.astype(np.float64)
            )
            mo = meanV @ WO_w.astype(np.float64).T + WO_b.astype(np.float64)
            full[mrows, :] = mo[None, :]
        out[b] = full.astype(np.float32)
    return out


# revision 17
# speedup vs baseline: 3.5754x; 1.1057x over previous
"""Multi-head attention TRN2 kernel (8 NeuronCores).

Sharding: batch (2) x head-group (4) data/tensor parallel -> 8 cores.
Core c handles batch b = c // 4 and heads [4g, 4g+4) where g = c % 4
(E-dim slice Dg = [256*g, 256*g+256)).

Mask-driven compaction (host side): the reference zeroes attention
weights of masked KEYS exactly (softmax of -1e9 underflows to 0.0 in
f32), and rows for masked QUERIES are recomputed on the host (uniform
attention), so the device only processes gathered unmasked positions:
  q' = count(mask)  padded to a multiple of 128   (queries)
  k' = count(mask)  padded to a multiple of 128   (keys; zero-padded kv
       and zero ebT rows make padding exactly weightless)
For the bench mask (~50% ones) this quarters the attention work and
halves the projections.

Device computes, per core, using transposed layouts throughout:
  QT = (WQ[Dg]/8) @ xg.T          [256, q']   (1/8 = 1/sqrt(DK))
  KT = WK[Dg] @ kvg.T             [256, k']
  V  = kvg @ WV[Dg].T             [k', 256]  (+ ones column per head)
  ST = KT_h.T-blocks @ QT_h       [k, q] scores, transposed
  e  = exp(ST) * ebT              ebT = exp(attn_bias.T) gathered (host)
  U' = [V_h | 1].T @ e            rows 0..63 = unnorm. head out.T, row 64 = denom
  UN = U'[0:64] / denom
  out_partial = UN.T-blocks @ WO[:, Dg].T   [q', 1024]

Attention runs per q-block (<=512 wide) in two head-pair passes so PSUM
holds: 2 rotating score buffers (4 banks) + U' accumulator (2 banks) +
out-proj buffers (2 banks).  AV matmuls trail scores by TRAIL kt-tiles
to hide the exp->mul latency; the U'-normalization epilogue is spread
over the first units of the NEXT pass with its multiplies on the idle
GpSimd engine; the out-projection of the previous q-block interleaves
into the score stream so the PE never drains.

Host: shards/gathers/transposes inputs, folds scale+exp(bias); sums the
4 row-parallel WO partials per batch, adds WO_b, scatters rows back and
overwrites masked-query rows with the uniform-attention value.
"""

import math
from contextlib import ExitStack

import ml_dtypes
import numpy as np

import concourse.bass as bass
import concourse.bacc as bacc
import concourse.tile as tile
from concourse import mybir
from concourse.bass_utils import run_bass_kernel_spmd

F32 = mybir.dt.float32
F32R = mybir.dt.float32r
BF16 = mybir.dt.bfloat16
AF = mybir.ActivationFunctionType

B, S, E, H, DK = 2, 2048, 1024, 16, 64
NC = 8
DG = 256          # dims per core (4 heads x 64)
HPC = 4           # heads per core
P = 128
NET = E // P      # 8 contraction tiles over E
TRAIL = 3         # AV matmuls trail scores by this many kt tiles

TRACE = False
LAST_RESULTS = {}

_NC_CACHE = {}


def _blocks(n):
    """Split n chunks into blocks of 2..4 chunks (a single block may be 1)."""
    out = []
    while n > 4:
        take = 4 if n - 4 != 1 else 3
        out.append(take)
        n -= take
    out.append(n)
    return out


def _build(nq, nk):
    """nq, nk: number of 128-wide query / key chunks."""
    qblocks = _blocks(nq)
    kblocks = _blocks(nk)
    SQ, SK = nq * P, nk * P

    nc = bacc.Bacc("TRN2", target_bir_lowering=False, debug=False, num_devices=NC)
    # chunk-granular tilings: [chunk, et, P, 128]
    xT = nc.dram_tensor("xT", [nq, NET, P, P], BF16, kind="ExternalInput").ap()
    kvT = nc.dram_tensor("kvT", [nk, NET, P, P], BF16, kind="ExternalInput").ap()
    wqT = nc.dram_tensor("wqT", [E, DG], BF16, kind="ExternalInput").ap()
    wkT = nc.dram_tensor("wkT", [E, DG], BF16, kind="ExternalInput").ap()
    wvT = nc.dram_tensor("wvT", [E, DG], BF16, kind="ExternalInput").ap()
    bq = nc.dram_tensor("bq", [DG], F32, kind="ExternalInput")
    bk = nc.dram_tensor("bk", [DG], F32, kind="ExternalInput")
    bv = nc.dram_tensor("bv", [DG], F32, kind="ExternalInput")
    # [ktile, P, qchunk, 128]
    ebT = nc.dram_tensor("ebT", [nk, P, nq, P], BF16, kind="ExternalInput").ap()
    r = nc.dram_tensor("r", [DG, E], BF16, kind="ExternalInput").ap()
    ones1 = nc.dram_tensor("ones1", [nk * HPC * P], BF16, kind="ExternalInput")
    # [qchunk, eb, P, 512]; host reassembles
    out = nc.dram_tensor("out", [nq, 2, P, 512], F32, kind="ExternalOutput").ap()

    wqv = wqT.rearrange("(t p) d -> p t d", p=P)
    wkv = wkT.rearrange("(t p) d -> p t d", p=P)
    wvv = wvT.rearrange("(t p) d -> p t d", p=P)

    with tile.TileContext(nc) as tc, ExitStack() as ctx:
        const = ctx.enter_context(tc.tile_pool(name="const", bufs=1))

        wq_sb = const.tile([P, NET, DG], BF16, name="wq_sb")
        wk_sb = const.tile([P, NET, DG], BF16, name="wk_sb")
        wv_sb = const.tile([P, NET, DG], BF16, name="wv_sb")
        vp_sb = const.tile([P, nk, HPC, P], BF16, name="vp_sb")
        bq_sb = const.tile([P, 2], F32, name="bq_sb")
        bk_sb = const.tile([P, 2], F32, name="bk_sb")
        bvb_sb = const.tile([P, DG], F32, name="bvb_sb")
        r_sb = const.tile([P, 2, E], BF16, name="r_sb")
        qt_sb = const.tile([P, 2, SQ], BF16, name="qt_sb")
        kt_sb = const.tile([P, 2, SK], BF16, name="kt_sb")
        un_sb = const.tile([P, 2, SQ], BF16, name="un_sb")

        # ebT double-buffered per q-block: [P, nk, 512] bf16
        ebpool = ctx.enter_context(tc.tile_pool(name="ebp", bufs=2))

        def emit_ebt_dma(ebt_tile, c0, bq_):
            for kt in range(nk):
                nc.sync.dma_start(
                    out=ebt_tile[:, kt, 0 : bq_ * P],
                    in_=ebT[kt, :, c0 : c0 + bq_].rearrange("p c q -> p (c q)"),
                )

        ebt_tiles = {}
        qstart = [0]
        for bsz in qblocks:
            qstart.append(qstart[-1] + bsz)

        # ---- Phase B: projections ----
        # DMA emission order is latency-critical: first matmuls need only the
        # first weight chunks and the first kv chunk; ebT/r stream later.
        with tc.tile_pool(
            name="xk", bufs=4 * max(len(kblocks), len(qblocks))
        ) as xkpool, tc.tile_pool(name="pj_ps", bufs=1, space="PSUM") as pj:
            # emit ALL input DMAs first (kv blocks, then x blocks), weights
            # interleaved so the first matmuls' dependencies land first
            kvt_tiles, xt_tiles = {}, {}
            c0 = 0
            for bi, bsz in enumerate(kblocks):
                for eg in range(4):
                    es2 = slice(eg * 2, (eg + 1) * 2)
                    if bi == 0:
                        nc.sync.dma_start(out=wk_sb[:, es2], in_=wkv[:, es2])
                        nc.sync.dma_start(out=wv_sb[:, es2], in_=wvv[:, es2])
                    kvt2 = xkpool.tile([P, 2, 512], BF16, tag="kvt", name="kvt2")
                    kvt_tiles[(bi, eg)] = kvt2
                    for ei in range(2):
                        nc.sync.dma_start(
                            out=kvt2[:, ei, 0 : bsz * P].rearrange(
                                "p (c q) -> p c q", c=bsz
                            ),
                            in_=kvT[c0 : c0 + bsz, eg * 2 + ei].rearrange(
                                "c p q -> p c q"
                            ),
                        )
                    if bi == 0 and eg == 0:
                        nc.sync.dma_start(
                            out=bq_sb, in_=bq.ap().rearrange("(t p) -> p t", p=P)
                        )
                        nc.sync.dma_start(
                            out=bk_sb, in_=bk.ap().rearrange("(t p) -> p t", p=P)
                        )
                        nc.sync.dma_start(
                            out=bvb_sb,
                            in_=bass.AP(tensor=bv, offset=0, ap=[[0, P], [1, DG]]),
                        )
                        nc.sync.dma_start(
                            out=vp_sb.rearrange("p a b c -> p (a b c)"),
                            in_=bass.AP(
                                tensor=ones1, offset=0,
                                ap=[[0, P], [1, nk * HPC * P]],
                            ),
                        )
                c0 += bsz
            c0 = 0
            for bi, bsz in enumerate(qblocks):
                for eg in range(4):
                    es2 = slice(eg * 2, (eg + 1) * 2)
                    if bi == 0:
                        nc.sync.dma_start(out=wq_sb[:, es2], in_=wqv[:, es2])
                    xt2 = xkpool.tile([P, 2, 512], BF16, tag="xt", name="xt2")
                    xt_tiles[(bi, eg)] = xt2
                    for ei in range(2):
                        nc.sync.dma_start(
                            out=xt2[:, ei, 0 : bsz * P].rearrange(
                                "p (c q) -> p c q", c=bsz
                            ),
                            in_=xT[c0 : c0 + bsz, eg * 2 + ei].rearrange(
                                "c p q -> p c q"
                            ),
                        )
                c0 += bsz
            # K/V projections over k-blocks
            c0 = 0
            for bi, bsz in enumerate(kblocks):
                ks = slice(c0 * P, (c0 + bsz) * P)
                ps_k = [pj.tile([P, 512], F32, tag=f"psk{d}", name=f"psk{d}") for d in range(2)]
                ps_v = [pj.tile([P, DG], F32, tag=f"psv{k}", name=f"psv{k}") for k in range(4)]
                for eg in range(4):
                    kvt2 = kvt_tiles[(bi, eg)]
                    for ei in range(2):
                        et = eg * 2 + ei
                        kvt = kvt2[:, ei, 0 : bsz * P]
                        st, sp = (et == 0), (et == NET - 1)
                        for d in range(2):
                            nc.tensor.matmul(
                                ps_k[d][:, 0 : bsz * P],
                                wk_sb[:, et, d * P : (d + 1) * P], kvt,
                                start=st, stop=sp,
                            )
                        for kb in range(bsz):
                            nc.tensor.matmul(
                                ps_v[kb], kvt[:, kb * P : (kb + 1) * P],
                                wv_sb[:, et, :], start=st, stop=sp,
                            )
                for d in range(2):
                    nc.vector.tensor_scalar_add(
                        kt_sb[:, d, ks], ps_k[d][:, 0 : bsz * P], bk_sb[:, d : d + 1]
                    )
                for kb in range(bsz):
                    nc.vector.tensor_add(
                        vp_sb[:, c0 + kb, :, DK : 2 * DK],
                        ps_v[kb].rearrange("p (h d) -> p h d", h=HPC),
                        bvb_sb.rearrange("p (h d) -> p h d", h=HPC),
                    )
                if bi == 0:
                    ebt_tiles[0] = ebpool.tile([P, nk, 512], BF16, tag="ebt", name="ebt0")
                    emit_ebt_dma(ebt_tiles[0], 0, qblocks[0])
                    nc.sync.dma_start(
                        out=r_sb, in_=r.rearrange("(t p) e -> p t e", p=P)
                    )
                c0 += bsz
            # Q projections over q-blocks
            c0 = 0
            for bi, bsz in enumerate(qblocks):
                qs = slice(c0 * P, (c0 + bsz) * P)
                ps_q = [pj.tile([P, 512], F32, tag=f"psq{d}", name=f"psq{d}") for d in range(2)]
                for eg in range(4):
                    xt2 = xt_tiles[(bi, eg)]
                    for ei in range(2):
                        et = eg * 2 + ei
                        xt = xt2[:, ei, 0 : bsz * P]
                        st, sp = (et == 0), (et == NET - 1)
                        for d in range(2):
                            nc.tensor.matmul(
                                ps_q[d][:, 0 : bsz * P],
                                wq_sb[:, et, d * P : (d + 1) * P], xt,
                                start=st, stop=sp,
                            )
                for d in range(2):
                    nc.vector.tensor_scalar_add(
                        qt_sb[:, d, qs], ps_q[d][:, 0 : bsz * P], bq_sb[:, d : d + 1]
                    )
                c0 += bsz

        # ---- Phase C: attention + interleaved out-projection ----
        with tc.tile_pool(name="fp", bufs=3) as fpool, tc.tile_pool(
            name="ep", bufs=TRAIL + 3
        ) as epool, tc.tile_pool(name="dn", bufs=2) as dpool, tc.tile_pool(
            name="osb", bufs=3
        ) as opool, tc.tile_pool(
            name="s_ps", bufs=2, space="PSUM"
        ) as sps, tc.tile_pool(
            name="u_ps", bufs=1, space="PSUM"
        ) as ups, tc.tile_pool(name="o_ps", bufs=2, space="PSUM") as ops:

            def d_unit(qchunk, eb):
                """Out-projection for one (128-query chunk, 512-col half)."""
                rs = slice(qchunk * P, (qchunk + 1) * P)
                es = slice(eb * 512, (eb + 1) * 512)
                ps_o = ops.tile([P, 512], F32, tag="pso", name="pso")
                for d in range(2):
                    nc.tensor.matmul(
                        ps_o, un_sb[:, d, rs], r_sb[:, d, es],
                        start=(d == 0), stop=(d == 1),
                    )
                osb = opool.tile([P, 512], F32, tag="osb", name="osb")
                if eb == 0:
                    nc.scalar.activation(osb, ps_o, AF.Copy)
                else:
                    nc.vector.tensor_copy(osb, ps_o)
                nc.sync.dma_start(out=out[qchunk, eb], in_=osb)

            def epi_step(st, step):
                """One piece of a finished pass's deferred epilogue, spread
                over the next pass so no engine queue blocks for long.  V'
                carries 64 replicated ones-columns, so U' rows 64..127 hold
                64 copies of the denominator: the reciprocal runs partition-
                parallel and the normalizing multiply needs no broadcast."""
                w, hp, qs, u = st["w"], st["hp"], st["qs"], st["u"]
                if step == 0:
                    rd = dpool.tile([DK, 2, 512], F32, tag="rd", name="rd")
                    nc.vector.reciprocal_approx_fast(
                        rd[:, :, 0:w], u[0:DK, :, 0:w]
                    )
                    st["rd"] = rd
                    un = dpool.tile([DK, 2, 512], F32, tag="unum", name="unum")
                    for j in range(2):
                        nc.sync.dma_start(
                            out=un[:, j, 0:w], in_=u[DK : 2 * DK, j, 0:w]
                        )
                    st["un"] = un
                elif step in (1, 2):
                    j = step - 1
                    nc.vector.tensor_mul(
                        un_sb[j * DK : (j + 1) * DK, hp, qs],
                        st["un"][:, j, 0:w], st["rd"][:, j, 0:w],
                    )

            epi_pend = None  # (state dict, emitted-steps)
            d_pend = []      # (qchunk, eb) out-proj units awaiting a slot

            for qi, bsz in enumerate(qblocks):
                c0 = qstart[qi]
                w = bsz * P
                qs = slice(c0 * P, (c0 + bsz) * P)
                ebt = ebt_tiles.pop(qi)
                for hp in range(2):  # head pair: heads {2*hp, 2*hp+1}
                    ps_u = ups.tile([2 * DK, 2, 512], F32, tag="psu", name="psu")
                    pend = []
                    for kt2 in range(nk):
                        if epi_pend is not None and kt2 in (0, 2, 3):
                            epi_step(epi_pend, 0 if kt2 == 0 else kt2 - 1)
                            if kt2 == 3:
                                epi_pend = None
                        if d_pend and kt2 >= 4:
                            d_unit(*d_pend.pop(0))
                        if hp == 0 and kt2 == 2 and qi + 1 < len(qblocks):
                            ebt_tiles[qi + 1] = ebpool.tile(
                                [P, nk, 512], BF16, tag="ebt", name="ebt1"
                            )
                            emit_ebt_dma(ebt_tiles[qi + 1], qstart[qi + 1], qblocks[qi + 1])
                        ks = slice(kt2 * P, (kt2 + 1) * P)
                        ps_s = sps.tile([P, 2, 512], F32, tag="pss", name="pss")
                        for j in range(2):
                            po = j * DK
                            nc.tensor.matmul(
                                ps_s[:, j, 0:w], kt_sb[po : po + DK, hp, ks],
                                qt_sb[po : po + DK, hp, qs], start=True, stop=True,
                            )
                        f2 = fpool.tile([P, 2, 512], BF16, tag="f", name="f2")
                        nc.scalar.activation(
                            f2[:, :, 0:w], ps_s[:, :, 0:w], AF.Exp
                        )
                        e2 = epool.tile([P, 2, 512], BF16, tag="e", name="e2")
                        for j in range(2):
                            nc.vector.tensor_mul(
                                e2[:, j, 0:w], f2[:, j, 0:w], ebt[:, kt2, 0:w]
                            )
                        pend.append((kt2, e2))
                        if len(pend) > TRAIL:
                            pkt, pe2 = pend.pop(0)
                            for j in range(2):
                                nc.tensor.matmul(
                                    ps_u[:, j, 0:w], vp_sb[:, pkt, 2 * hp + j, :],
                                    pe2[:, j, 0:w], start=(pkt == 0), stop=False,
                                )
                    for idx, (pkt, pe2) in enumerate(pend):
                        last = idx == len(pend) - 1
                        for j in range(2):
                            nc.tensor.matmul(
                                ps_u[:, j, 0:w], vp_sb[:, pkt, 2 * hp + j, :],
                                pe2[:, j, 0:w], start=(pkt == 0), stop=last,
                            )
                    # evict U' now (frees the PSUM accumulator); the rest of
                    # the epilogue is spread over the next pass's units.
                    u_raw = dpool.tile([2 * DK, 2, 512], F32, tag="uraw", name="u_raw")
                    nc.scalar.activation(u_raw[:, :, 0:w], ps_u[:, :, 0:w], AF.Copy)
                    if epi_pend is not None:  # nk < 4: flush leftovers
                        for step in range(3):
                            epi_step(epi_pend, step)
                    epi_pend = {"u": u_raw, "hp": hp, "qs": qs, "w": w}
                    if hp == 1:
                        d_pend.extend((c0 + c, eb) for c in range(bsz) for eb in range(2))
            # tail: last pass's epilogue + remaining out-proj units
            for step in range(3):
                epi_step(epi_pend, step)
            for unit in d_pend:
                d_unit(*unit)

    nc.compile()
    return nc


def _get_nc(nq, nk):
    key = (nq, nk)
    if key not in _NC_CACHE:
        _NC_CACHE[key] = _build(nq, nk)
    return _NC_CACHE[key]


def kernel(x, kv, mask, attn_bias, WQ_w, WQ_b, WK_w, WK_b, WV_w, WV_b, WO_w, WO_b):
    x = np.asarray(x, dtype=np.float32)
    kv = np.asarray(kv, dtype=np.float32)
    mask = np.asarray(mask)
    attn_bias = np.asarray(attn_bias, dtype=np.float32)
    WQ_w = np.asarray(WQ_w, dtype=np.float32)
    WQ_b = np.asarray(WQ_b, dtype=np.float32)
    WK_w = np.asarray(WK_w, dtype=np.float32)
    WK_b = np.asarray(WK_b, dtype=np.float32)
    WV_w = np.asarray(WV_w, dtype=np.float32)
    WV_b = np.asarray(WV_b, dtype=np.float32)
    WO_w = np.asarray(WO_w, dtype=np.float32)
    WO_b = np.asarray(WO_b, dtype=np.float32)

    sc = 1.0 / math.sqrt(DK)
    maskf = mask.astype(np.float32)

    idxs = [np.nonzero(mask[b])[0] for b in range(B)]
    counts = [len(ix) for ix in idxs]
    nq = nk = max(1, max((c + P - 1) // P for c in counts))
    SQ = SK = nq * P

    def _tile_chunks(aT, n):
        # [E, n*P] -> [n, E//P, P, P]
        return np.ascontiguousarray(
            aT.reshape(NET, P, n, P).transpose(2, 0, 1, 3)
        )

    xTs, kvTs, ebTs = [], [], []
    for b in range(B):
        ix = idxs[b]
        xg = np.zeros((SQ, E), np.float32)
        xg[: counts[b]] = x[b][ix]
        kvg = np.zeros((SK, E), np.float32)
        kvg[: counts[b]] = kv[b][ix]
        ebg = np.zeros((SK, SQ), np.float32)
        ebg[: counts[b], : counts[b]] = np.exp(attn_bias[b][np.ix_(ix, ix)]).T
        # padded query columns: give them one nonzero weight so their
        # denominator is finite (results are discarded by the scatter)
        ebg[0, counts[b] :] = 1.0
        xTs.append(_tile_chunks(xg.T.astype(ml_dtypes.bfloat16), nq))
        kvTs.append(_tile_chunks(kvg.T.astype(ml_dtypes.bfloat16), nk))
        # [SK, SQ] -> [nk, P, nq, P]
        ebTs.append(
            np.ascontiguousarray(
                ebg.astype(ml_dtypes.bfloat16)
                .reshape(nk, P, nq, P)
            )
        )

    in_maps = []
    for c in range(NC):
        b, g = c // 4, c % 4
        Dg = slice(DG * g, DG * (g + 1))
        in_maps.append(
            {
                "xT": xTs[b],
                "kvT": kvTs[b],
                "wqT": np.ascontiguousarray((WQ_w[Dg] * sc).T.astype(ml_dtypes.bfloat16)),
                "wkT": np.ascontiguousarray(WK_w[Dg].T.astype(ml_dtypes.bfloat16)),
                "wvT": np.ascontiguousarray(WV_w[Dg].T.astype(ml_dtypes.bfloat16)),
                "bq": np.ascontiguousarray(WQ_b[Dg] * sc),
                "bk": np.ascontiguousarray(WK_b[Dg]),
                "bv": np.ascontiguousarray(WV_b[Dg]),
                "ebT": ebTs[b],
                "r": np.ascontiguousarray(WO_w[:, Dg].T.astype(ml_dtypes.bfloat16)),
                "ones1": np.ones(nk * HPC * P, ml_dtypes.bfloat16),
            }
        )

    nc = _get_nc(nq, nk)
    res = run_bass_kernel_spmd(nc, in_maps, list(range(NC)), trace=TRACE)
    LAST_RESULTS["res"] = res

    out = np.zeros((B, S, E), np.float32)
    for b in range(B):
        acc = np.zeros((SQ, E), np.float64)
        for g in range(4):
            ot = res.results[b * 4 + g]["out"]  # [nq, 2, P, 512]
            acc += ot.transpose(0, 2, 1, 3).reshape(SQ, E).astype(np.float64)
        acc += WO_b.astype(np.float64)[None, :]
        full = np.zeros((S, E), np.float64)
        full[idxs[b]] = acc[: counts[b]]
        # masked-query rows: reference softmax of an all(-1e9) row is uniform
        mrows = maskf[b] == 0.0
        if mrows.any():
            meanV = (
                kv[b].astype(np.float64).mean(axis=0) @ WV_w.astype(np.float64).T
                + WV_b.astype(np.float64)
            )
            mo = meanV @ WO_w.astype(np.float64).T + WO_b.astype(np.float64)
            full[mrows, :] = mo[None, :]
        out[b] = full.astype(np.float32)
    return out
